# revision 46
# baseline (speedup 1.0000x reference)
"""Trainium2 Bass kernel for the 4-layer quantized strided CNN.

Strategy:
  - Pure data parallelism: 8 cores = 4 batch x 2 H-halves. One uniform SPMD
    program; per-core differences enter only through input data (shards +
    mask planes).
  - Forward-pass identity: sum_i floor((round(w)+i)/s) == round(w), so the
    split-loop qconv collapses to ONE conv with integer weights round(w) and
    bias round(b). All arithmetic on integers < 2^24 is exact in f32/fp16.
  - Activations/weights stored fp16 (integers up to 2048 exact), matmul on
    the PE at 1 cycle/row with fp32 PSUM accumulation -> bit-exact vs the
    f32 jax reference.
  - L1 (Cin=3): weights-stationary, im2col activations (built via DMA
    gathers from a parity-split DRAM copy of the quantized input).
  - L2..L4 (Cin=192): activations-stationary [K=cin-chunk, M=128 pixels],
    moving weights [K, Cout]; conv = 25 taps x 2 K-chunks accumulated in
    PSUM. Bias is folded into the matmul via a ones/mask row appended to the
    K=64 tail chunk; the mask row also zeroes out-of-image rows so they
    requantize to exactly 0.
  - Requant chain is bit-exact: floor(v) = i32cast(v) - (i32cast(v) > v)
    (HW f32->i32 cast is round-half-even, verified), pow2 scalings commute
    with fp32 rounding.
  - PE transposes (via identity) convert [pix, cout] back to channel-major
    for the next layer. L4 output stays [pix, cout]; host transposes.
"""

import numpy as np

import concourse.bass as bass
import concourse.bacc as bacc
import concourse.mybir as mybir
import concourse.tile as tile
from concourse.bass_utils import run_bass_kernel_spmd
from concourse.masks import make_identity

F32 = mybir.dt.float32
F16 = mybir.dt.float16
I32 = mybir.dt.int32
AOP = mybir.AluOpType

N_CORES = 8
CLP_K = 7
IN_SCALE = 8


class Cfg:
    """Geometry for the uniform per-core program."""

    def __init__(self, H=512, W=512, rows4=16):
        self.H, self.W = H, W
        self.r4 = rows4                    # L4 out rows per core
        self.r3 = 2 * rows4 + 3            # L3 out slots (window)
        self.r2 = 4 * rows4 + 9            # L2 out slots
        self.r1 = 8 * rows4 + 21           # L1 out slots
        self.rx = 16 * rows4 + 45          # x rows per shard
        self.w1o = W // 2
        self.w2o = W // 4
        self.w3o = W // 8
        self.w4o = W // 16
        self.fr1, self.fr2 = H // 2, H // 4
        self.fr3, self.fr4 = H // 8, H // 16
        self.rx_half = (self.rx + 1) // 2


def build_program(cfg: Cfg, detect_races=True, fast=True):
    nc = bacc.Bacc("TRN2", target_bir_lowering=False, debug=False,
                   num_devices=N_CORES,
                   detect_race_conditions=detect_races)

    WX = cfg.W + 4
    W1P = cfg.w1o + 4
    W2P = cfg.w2o + 4
    W3P = cfg.w3o + 4

    # ---------------- parameters ----------------
    x_h = None  # declared below once nrows_pad is known
    w1_h = nc.declare_dram_parameter("w1m", [76, 192], F16, isOutput=False)
    w2a_h = nc.declare_dram_parameter("w2a", [128, 25, 192], F16, isOutput=False)
    w2p_h = nc.declare_dram_parameter("w2p", [128, 10, 192], F16, isOutput=False)
    w2l_h = nc.declare_dram_parameter("w2l", [65, 5, 192], F16, isOutput=False)
    w3a_h = nc.declare_dram_parameter("w3a", [128, 25, 192], F16, isOutput=False)
    w3p_h = nc.declare_dram_parameter("w3p", [128, 10, 192], F16, isOutput=False)
    w3l_h = nc.declare_dram_parameter("w3l", [65, 5, 192], F16, isOutput=False)
    w4a_h = nc.declare_dram_parameter("w4a", [128, 25, 320], F16, isOutput=False)
    w4p_h = nc.declare_dram_parameter("w4p", [128, 10, 320], F16, isOutput=False)
    w4l_h = nc.declare_dram_parameter("w4l", [65, 5, 320], F16, isOutput=False)
    m1_h = nc.declare_dram_parameter("m1", [128, 2], F32, isOutput=False)
    m2_h = nc.declare_dram_parameter("m2", [192], F32, isOutput=False)
    m3_h = nc.declare_dram_parameter("m3", [128, 2], F32, isOutput=False)
    m4_h = nc.declare_dram_parameter("m4", [128, 3], F32, isOutput=False)
    sc_h = nc.declare_dram_parameter("sc", [12], F32, isOutput=False)
    mp2_h = nc.declare_dram_parameter("mp2", [cfg.r1, W1P], F16, isOutput=False)
    mp3_h = nc.declare_dram_parameter("mp3", [cfg.r2, W2P], F16, isOutput=False)
    mp4_h = nc.declare_dram_parameter("mp4", [cfg.r3, W3P], F16, isOutput=False)
    out_h = nc.declare_dram_parameter("out", [320, cfg.r4 * cfg.w4o], F32,
                                      isOutput=True)

    x_h = nc.declare_dram_parameter(
        "x", [((76 * cfg.rx_half + 127) // 128) * 128, WX // 2], F32,
        isOutput=False)
    xq_h = nc.dram_tensor(
        "xq_par", [((76 * cfg.rx_half + 127) // 128) * 128, WX // 2], F16)
    RB1 = 38
    x2_bounds = list(range(0, cfg.r1, RB1)) + [cfg.r1]
    x2s_h = [nc.dram_tensor(f"x2s{k}",
                            [193, x2_bounds[k + 1] - x2_bounds[k], W1P], F16)
             for k in range(len(x2_bounds) - 1)]

    nrows_flat = 76 * cfg.rx_half
    rows_pp = (nrows_flat + 127) // 128       # flat rows per partition
    nrows_pad = rows_pp * 128

    with tile.TileContext(nc) as tc:
        consts_cm = tc.tile_pool(name="consts", bufs=1)
        consts = consts_cm.__enter__()

        ident = consts.tile([128, 128], F16)
        make_identity(nc, ident)

        def load(h, shape, dt=F16, tag=None):
            t = consts.tile(shape, dt, tag=tag)
            nc.sync.dma_start(out=t, in_=h[:])
            return t

        w1sb = load(w1_h, [76, 192], tag="w1sb")
        t3t = consts.tile([128, cfg.r2, W2P], F16, tag="t3t")
        t4t = consts.tile([128, cfg.r3, W3P], F16, tag="t4t")
        m1sb = load(m1_h, [128, 2], F32, tag="m1sb")

        def bcast_tile(src_h, n, tag):
            t = consts.tile([128, n], F32, tag=tag)
            nc.sync.dma_start(out=t, in_=bass.AP(src_h, 0, [[0, 128], [1, n]]))
            return t

        scbc = bcast_tile(sc_h, 12, "scbc")
        half_col = consts.tile([128, 1], F32)
        nc.vector.memset(half_col, 0.5)

        x3a = consts.tile([128, cfg.r2, W2P], F16)
        x3b = consts.tile([65, cfg.r2, W2P], F16)
        x4a = consts.tile([128, cfg.r3, W3P], F16)
        x4b = consts.tile([65, cfg.r3, W3P], F16)
        for t_, wp in ((x3a, W2P), (x3b, W2P), (x4a, W3P), (x4b, W3P)):
            nc.vector.memset(t_[:, :, 0:2], 0.0)
            nc.vector.memset(t_[:, :, wp - 2:wp], 0.0)
        nc.sync.dma_start(out=x3b[64:65, :, :], in_=mp3_h[:])
        nc.sync.dma_start(out=x4b[64:65, :, :], in_=mp4_h[:])

        # =========== input quantization: xq = clip(rhe(x*256),0,255) ===========
        # x arrives host-parity-split + padded; quant = one elementwise pass.
        # Partition p holds flat rows [p*rows_pp, (p+1)*rows_pp).
        WH = WX // 2
        fpp = rows_pp * WH
        NQC = max(1, (fpp * 20 + 84999) // 85000)  # chunk to fit SBUF
        qc = (fpp + NQC - 1) // NQC
        with tc.tile_pool(name="quant", bufs=2) as qpool:
            for ci_ in range(NQC):
                f0 = ci_ * qc
                fw = min(qc, fpp - f0)
                eng_in = nc.sync if ci_ % 2 == 0 else nc.scalar
                eng_out = nc.scalar if ci_ % 2 == 0 else nc.sync
                xin = qpool.tile([128, qc], F32, tag="xin")
                eng_in.dma_start(
                    out=xin[:, :fw],
                    in_=bass.AP(x_h, f0, [[fpp, 128], [1, fw]]))
                ti = qpool.tile([128, qc], I32, tag="ti")
                nc.vector.tensor_scalar(ti[:, :fw], xin[:, :fw], 256.0, None,
                                        AOP.mult)
                xqt = qpool.tile([128, qc], F16, tag="xqt")
                nc.gpsimd.tensor_scalar(xqt[:, :fw], ti[:, :fw], 0.0, 255.0,
                                        AOP.max, AOP.min)
                eng_out.dma_start(
                    out=bass.AP(xq_h, f0, [[fpp, 128], [1, fw]]),
                    in_=xqt[:, :fw])
        # x2 mask plane 192 <- mp2 (per split tensor)
        for k in range(len(x2s_h)):
            b0, b1 = x2_bounds[k], x2_bounds[k + 1]
            nc.scalar.dma_start(
                out=bass.AP(x2s_h[k], 192 * (b1 - b0) * W1P,
                            [[W1P, b1 - b0], [1, W1P]]),
                in_=bass.AP(mp2_h, b0 * W1P, [[W1P, b1 - b0], [1, W1P]]))

        # ============================ Layer 1 ============================
        STG = 8
        with tc.tile_pool(name="l1R", bufs=2) as rpool, \
             tc.tile_pool(name="l1ps", bufs=3, space="PSUM") as pspool, \
             tc.tile_pool(name="l1t", bufs=2) as tpool, \
             tc.tile_pool(name="l1s", bufs=4) as spool:

            def _l1_pair(ci, ca, cb, cw, R, j, jw, st, sr):
                ps = pspool.tile([128, 4, cfg.w1o], F32, tag="ps")
                for mj in range(0, jw, 2):
                    mw = min(2, jw - mj)
                    nc.tensor.matmul(
                        ps[:cw, mj:mj + mw, :], w1sb[:, ca:cb],
                        R[:, j + mj:j + mj + mw, 0:cfg.w1o],
                        start=True, stop=True)
                s = tpool.tile([128, 4, cfg.w1o], F32, tag="s")
                nc.scalar.activation(
                    s[:cw, :jw, :], ps[:cw, :jw, :],
                    mybir.ActivationFunctionType.Identity,
                    bias=half_col[0:cw, :], scale=m1sb[0:cw, ci:ci + 1])
                dst = st[:cw, sr:sr + jw, 2:2 + cfg.w1o]
                if fast:
                    v = tpool.tile([128, 4, cfg.w1o], F32, tag="v")
                    nc.vector.tensor_scalar(
                        v[:cw, :jw, :], s[:cw, :jw, :],
                        scbc[0:cw, 0:1], scbc[0:cw, 3:4], AOP.min, AOP.mult)
                    ti2 = tpool.tile([128, 4, cfg.w1o], I32, tag="ti2")
                    nc.gpsimd.tensor_copy(ti2[:cw, :jw, :], v[:cw, :jw, :])
                    if (j // 4) % 3 == 0:
                        nc.gpsimd.tensor_copy(dst, ti2[:cw, :jw, :])
                    else:
                        nc.vector.tensor_copy(dst, ti2[:cw, :jw, :])
                    return
                s2 = tpool.tile([128, 4, cfg.w1o], F32, tag="s2")
                nc.vector.tensor_scalar(
                    s2[:cw, :jw, :], s[:cw, :jw, :],
                    0.0, scbc[0:cw, 0:1], AOP.max, AOP.min)
                ti1 = tpool.tile([128, 4, cfg.w1o], I32, tag="ti1")
                nc.gpsimd.tensor_copy(ti1[:cw, :jw, :], s2[:cw, :jw, :])
                g1 = tpool.tile([128, 4, cfg.w1o], F32, tag="g1")
                nc.gpsimd.tensor_tensor(
                    g1[:cw, :jw, :], ti1[:cw, :jw, :], s2[:cw, :jw, :],
                    AOP.is_gt)
                c1t = tpool.tile([128, 4, cfg.w1o], F32, tag="c1t")
                nc.vector.tensor_tensor(
                    c1t[:cw, :jw, :], ti1[:cw, :jw, :], g1[:cw, :jw, :],
                    AOP.subtract)
                v = tpool.tile([128, 4, cfg.w1o], F32, tag="v")
                nc.vector.tensor_scalar(
                    v[:cw, :jw, :], c1t[:cw, :jw, :],
                    scbc[0:cw, 3:4], 0.5, AOP.mult, AOP.add)
                ti2 = tpool.tile([128, 4, cfg.w1o], I32, tag="ti2")
                nc.gpsimd.tensor_copy(ti2[:cw, :jw, :], v[:cw, :jw, :])
                g2 = tpool.tile([128, 4, cfg.w1o], F32, tag="g2")
                nc.gpsimd.tensor_tensor(
                    g2[:cw, :jw, :], ti2[:cw, :jw, :], v[:cw, :jw, :],
                    AOP.is_gt)
                nc.vector.tensor_tensor(
                    dst, ti2[:cw, :jw, :], g2[:cw, :jw, :], AOP.subtract)

            wload = {}
            n_blk = (cfg.r1 + RB1 - 1) // RB1
            for blk in range(n_blk):
                j0 = blk * RB1
                nj = min(RB1, cfg.r1 - j0)
                R = rpool.tile([76, RB1, WX // 2], F16, tag="R")
                nc.sync.dma_start(
                    out=R[:, :nj, :],
                    in_=bass.AP(xq_h, j0 * WH,
                                [[cfg.rx_half * WH, 76], [1, nj * WH]]))
                if blk == 0:
                    wload[0] = (load(w2a_h, [128, 25, 192], tag="w2a"),
                                load(w2p_h, [128, 10, 192], tag="w2p"),
                                load(w2l_h, [65, 5, 192], tag="w2l"),
                                bcast_tile(m2_h, 192, "m2bc"))
                elif blk == 1:
                    wload[1] = (load(w3a_h, [128, 25, 192], tag="w3a"),
                                load(w3p_h, [128, 10, 192], tag="w3p"),
                                load(w3l_h, [65, 5, 192], tag="w3l"),
                                load(w4a_h, [128, 25, 320], tag="w4a"),
                                load(w4p_h, [128, 10, 320], tag="w4p"),
                                load(w4l_h, [65, 5, 320], tag="w4l"),
                                load(m3_h, [128, 2], F32, tag="m3sb"),
                                load(m4_h, [128, 3], F32, tag="m4sb"))

                for ci, (ca, cb) in enumerate(((0, 128), (128, 192))):
                    cw = cb - ca
                    for g0 in range(0, nj, STG):
                        gw = min(STG, nj - g0)
                        st = spool.tile([128, STG, W1P], F16, tag="st")
                        nc.vector.memset(st[:cw, :gw, 0:2], 0.0)
                        nc.vector.memset(st[:cw, :gw, W1P - 2:W1P], 0.0)
                        for j in range(g0, g0 + gw, 4):
                            jw = min(4, g0 + gw - j)
                            _l1_pair(ci, ca, cb, cw, R, j, jw, st, j - g0)
                        rk_ = x2_bounds[blk + 1] - x2_bounds[blk]
                        nc.scalar.dma_start(
                            out=bass.AP(x2s_h[blk],
                                        (ca * rk_ + (j0 + g0 -
                                                     x2_bounds[blk])) * W1P,
                                        [[rk_ * W1P, cw], [W1P, gw],
                                         [1, W1P]]),
                            in_=st[:cw, :gw, :])

        # =================== requant for [pix, cout] layout ===================
        def requant_full(q_ps, pw, cout, mbc, clp_col, scl_col, c5s_col,
                         tpool, tag):
            t1 = tpool.tile([128, cout], F32, tag=tag + "t1")
            nc.vector.tensor_tensor(t1[:pw], q_ps[:pw], mbc[:pw], AOP.mult)
            qf = tpool.tile([128, cout], F16, tag=tag + "qf")
            if fast:
                s = tpool.tile([128, cout], F32, tag=tag + "s")
                nc.scalar.activation(s[:pw], t1[:pw],
                                     mybir.ActivationFunctionType.Identity,
                                     bias=half_col[:pw, :], scale=1.0)
                v = tpool.tile([128, cout], F32, tag=tag + "v")
                nc.vector.tensor_scalar(v[:pw], s[:pw], clp_col[:pw],
                                        scl_col[:pw], AOP.min, AOP.mult)
                ti2 = tpool.tile([128, cout], I32, tag=tag + "ti2")
                nc.gpsimd.tensor_copy(ti2[:pw], v[:pw])
                nc.gpsimd.tensor_copy(qf[:pw], ti2[:pw])
                return qf
            s = tpool.tile([128, cout], F32, tag=tag + "s")
            nc.vector.tensor_scalar(s[:pw], t1[:pw], 0.5, 0.0,
                                    AOP.add, AOP.max)
            s2 = tpool.tile([128, cout], F32, tag=tag + "s2")
            nc.vector.tensor_scalar(s2[:pw], s[:pw], clp_col[:pw], None,
                                    AOP.min)
            ti1 = tpool.tile([128, cout], I32, tag=tag + "ti1")
            nc.gpsimd.tensor_copy(ti1[:pw], s2[:pw])
            g1 = tpool.tile([128, cout], F32, tag=tag + "g1")
            nc.gpsimd.tensor_tensor(g1[:pw], ti1[:pw], s2[:pw], AOP.is_gt)
            c1 = tpool.tile([128, cout], F32, tag=tag + "c1")
            nc.gpsimd.tensor_tensor(c1[:pw], ti1[:pw], g1[:pw], AOP.subtract)
            v = tpool.tile([128, cout], F32, tag=tag + "v")
            nc.vector.tensor_scalar(v[:pw], c1[:pw], scl_col[:pw], 0.5,
                                    AOP.mult, AOP.add)
            ti2 = tpool.tile([128, cout], I32, tag=tag + "ti2")
            nc.gpsimd.tensor_copy(ti2[:pw], v[:pw])
            g2 = tpool.tile([128, cout], F32, tag=tag + "g2")
            nc.gpsimd.tensor_tensor(g2[:pw], ti2[:pw], v[:pw], AOP.is_gt)
            nc.vector.tensor_tensor(qf[:pw], ti2[:pw], g2[:pw], AOP.subtract)
            return qf

        def conv_tiles(src_a, src_b, wa, wb, cout, n_out_rows, out_w,
                       rows_per_tile, pspool, emit_out):
            j = 0
            while j < n_out_rows:
                jw = min(rows_per_tile, n_out_rows - j)
                pw = jw * out_w
                ps = pspool.tile([128, cout], F32, tag="cps")
                first = True
                for ky in range(5):
                    for kx in range(5):
                        tap = ky * 5 + kx
                        last = (ky == 4 and kx == 4)
                        nc.tensor.matmul(ps[:pw], src_a(j, jw, ky, kx),
                                         wa[:, tap, :], start=first,
                                         stop=False)
                        first = False
                        nc.tensor.matmul(ps[:pw], src_b(j, jw, ky, kx),
                                         wb[:, tap, :], start=False, stop=last)
                emit_out(j, jw, ps, pw)
                j += jw

        w2a, w2p, w2l, m2bc = wload[0]
        if 1 not in wload:
            wload[1] = (load(w3a_h, [128, 25, 192], tag="w3a"),
                        load(w3p_h, [128, 10, 192], tag="w3p"),
                        load(w3l_h, [65, 5, 192], tag="w3l"),
                        load(w4a_h, [128, 25, 320], tag="w4a"),
                        load(w4p_h, [128, 10, 320], tag="w4p"),
                        load(w4l_h, [65, 5, 320], tag="w4l"),
                        load(m3_h, [128, 2], F32, tag="m3sb"),
                        load(m4_h, [128, 3], F32, tag="m4sb"))
        w3a, w3p, w3l, w4a, w4p, w4l, m3sb, m4sb = wload[1]

        # ============================ Layer 2 ============================
        # Tail-chunk pairing: cin 128..191 of taps (ky,ky+1) packed into one
        # K=128 contraction via a row-shifted tail tile T. 25 full + 10 pair
        # + 5 leftover = 40 matmuls/tile (vs 50).
        RB2 = 10
        with tc.tile_pool(name="l2r", bufs=2) as r2pool, \
             tc.tile_pool(name="l2ps", bufs=4, space="PSUM") as ps2, \
             tc.tile_pool(name="l2tr", bufs=2, space="PSUM") as tr2, \
             tc.tile_pool(name="l2t", bufs=2) as t2pool:
            n_blk = (cfg.r2 + RB2 - 1) // RB2
            for blk in range(n_blk):
                j0 = blk * RB2
                nj = min(RB2, cfg.r2 - j0)
                nin = 2 * nj + 3
                def x2_read(dst, d0, np_, pl0, gr0, nrows):
                    for k in range(len(x2s_h)):
                        b0, b1 = x2_bounds[k], x2_bounds[k + 1]
                        lo, hi = max(gr0, b0), min(gr0 + nrows, b1)
                        if hi > lo:
                            rk = b1 - b0
                            nc.sync.dma_start(
                                out=dst[d0:d0 + np_,
                                        lo - gr0:hi - gr0, :],
                                in_=bass.AP(
                                    x2s_h[k],
                                    (pl0 * rk + (lo - b0)) * W1P,
                                    [[rk * W1P, np_], [W1P, hi - lo],
                                     [1, W1P]]))

                ra = r2pool.tile([128, 2 * RB2 + 3, W1P], F16, tag="ra")
                x2_read(ra, 0, 128, 0, 2 * j0, nin)
                rb = r2pool.tile([65, 2 * RB2 + 3, W1P], F16, tag="rb")
                x2_read(rb, 0, 65, 128, 2 * j0, nin)
                # T: rows shifted pair tile (lower = tail row r, upper = r+1)
                tt = r2pool.tile([128, 2 * RB2 + 3, W1P], F16, tag="tt")
                x2_read(tt, 0, 64, 128, 2 * j0, nin)
                nup = min(nin, cfg.r1 - (2 * j0 + 1))
                x2_read(tt, 64, 64, 128, 2 * j0 + 1, nup)

                def emit2(j, ps, _j0=j0):
                    qf = requant_full(ps, 128, 192, m2bc, scbc[:, 1:2],
                                      scbc[:, 4:5], scbc[:, 7:8],
                                      t2pool, "l2")
                    trp = tr2.tile([128, 2, 128], F16, tag="trp")
                    nc.tensor.transpose(trp[:, 0, :], qf[:, 0:128], ident)
                    nc.tensor.transpose(trp[0:64, 1, :], qf[:, 128:192], ident)
                    jj = _j0 + j
                    nc.scalar.copy(x3a[:, jj, 2:2 + cfg.w2o], trp[:, 0, :])
                    nc.scalar.copy(x3b[0:64, jj, 2:2 + cfg.w2o],
                                   trp[0:64, 1, :])

                ce = 2 * cfg.w2o - 1
                for j in range(nj):
                    ps = ps2.tile([128, 192], F32, tag="cps")
                    first = True
                    for ky in range(5):
                        for kx in range(5):
                            nc.tensor.matmul(
                                ps[:], ra[0:128, 2 * j + ky, kx:kx + ce:2],
                                w2a[:, ky * 5 + kx, :],
                                start=first, stop=False)
                            first = False
                    for kyp in range(2):
                        for kx in range(5):
                            nc.tensor.matmul(
                                ps[:],
                                tt[0:128, 2 * j + 2 * kyp, kx:kx + ce:2],
                                w2p[:, kyp * 5 + kx, :],
                                start=False, stop=False)
                    for kx in range(5):
                        nc.tensor.matmul(
                            ps[:], rb[0:65, 2 * j + 4, kx:kx + ce:2],
                            w2l[:, kx, :], start=False, stop=(kx == 4))
                    emit2(j, ps)

        # ===== L3/L4: weights-stationary (stationary = [K, cout] 1 free dim),
        # moving = activations with 2D pixel APs; output lands channel-major.
        def requant_cm(q_ap, cw, mcol, c5s_col, sclB_col, clp_col,
                       pool, tag, dims, out_writer):
            """Channel-major requant: q [cw, *dims] psum -> fp16 via writer."""
            s = pool.tile([128] + dims, F32, tag=tag + "s")
            sl = (slice(0, cw),) + tuple(slice(0, d) for d in dims)
            nc.scalar.activation(s[sl], q_ap,
                                 mybir.ActivationFunctionType.Identity,
                                 bias=half_col[0:cw, :], scale=mcol)
            if fast:
                v = pool.tile([128] + dims, F32, tag=tag + "v")
                nc.vector.tensor_scalar(v[sl], s[sl], clp_col, sclB_col,
                                        AOP.min, AOP.mult)
                ti = pool.tile([128] + dims, I32, tag=tag + "ti")
                nc.gpsimd.tensor_copy(ti[sl], v[sl])
                out_writer(ti[sl])
                return
            s2 = pool.tile([128] + dims, F32, tag=tag + "s2")
            nc.vector.tensor_scalar(s2[sl], s[sl], 0.0, clp_col,
                                    AOP.max, AOP.min)
            ti1 = pool.tile([128] + dims, I32, tag=tag + "ti1")
            nc.gpsimd.tensor_copy(ti1[sl], s2[sl])
            g1 = pool.tile([128] + dims, F32, tag=tag + "g1")
            nc.gpsimd.tensor_tensor(g1[sl], ti1[sl], s2[sl], AOP.is_gt)
            c1 = pool.tile([128] + dims, F32, tag=tag + "c1")
            nc.vector.tensor_tensor(c1[sl], ti1[sl], g1[sl], AOP.subtract)
            v = pool.tile([128] + dims, F32, tag=tag + "v")
            nc.vector.tensor_scalar(v[sl], c1[sl], sclB_col, 0.5,
                                    AOP.mult, AOP.add)
            ti2 = pool.tile([128] + dims, I32, tag=tag + "ti2")
            nc.gpsimd.tensor_copy(ti2[sl], v[sl])
            g2 = pool.tile([128] + dims, F32, tag=tag + "g2")
            nc.gpsimd.tensor_tensor(g2[sl], ti2[sl], v[sl], AOP.is_gt)
            out_writer((ti2[sl], g2[sl]))

        def cm_write(dst_ap, res):
            if fast:
                nc.vector.tensor_copy(dst_ap, res)
            else:
                ti2, g2 = res
                nc.vector.tensor_tensor(dst_ap, ti2, g2, AOP.subtract)

        def conv_ws(wa, wp, wl, src_a, src_t, src_l, chunks, n_out_rows,
                    out_w, rpt, pspool, emit):
            # 25 full + 10 paired-tail + 5 leftover matmuls per psum
            j = 0
            while j < n_out_rows:
                jw = min(rpt, n_out_rows - j)
                for ci, (ca, cb) in enumerate(chunks):
                    cw = cb - ca
                    ps = pspool.tile([128, rpt, out_w], F32, tag="wps")
                    first = True
                    for ky in range(5):
                        for kx in range(5):
                            nc.tensor.matmul(
                                ps[:cw, :jw, :], wa[:, ky * 5 + kx, ca:cb],
                                src_a(j, jw, ky, kx), start=first, stop=False)
                            first = False
                    for kyp in range(2):
                        for kx in range(5):
                            nc.tensor.matmul(
                                ps[:cw, :jw, :], wp[:, kyp * 5 + kx, ca:cb],
                                src_t(j, jw, 2 * kyp, kx),
                                start=False, stop=False)
                    for kx in range(5):
                        nc.tensor.matmul(
                            ps[:cw, :jw, :], wl[:, kx, ca:cb],
                            src_l(j, jw, 4, kx), start=False, stop=(kx == 4))
                    emit(j, jw, ci, ca, cb, ps)
                j += jw

        # ============================ Layer 3 ============================
        RB3 = 8
        with tc.tile_pool(name="l3ps", bufs=4, space="PSUM") as ps3, \
             tc.tile_pool(name="l3t", bufs=2) as t3pool:

            # row-shifted tail pair tile for L3 (built after L2 completes)
            nc.sync.dma_start(out=t3t[0:64, :, :], in_=x3b[0:64, :, :])
            nc.sync.dma_start(out=t3t[64:128, 0:cfg.r2 - 1, :],
                              in_=x3b[0:64, 1:cfg.r2, :])

            def src_a3(j, jw, ky, kx):
                return x3a[0:128, 2 * j + ky:2 * j + ky + 2 * jw - 1:2,
                           kx:kx + 2 * cfg.w3o - 1:2]

            def src_t3(j, jw, ky, kx):
                return t3t[0:128, 2 * j + ky:2 * j + ky + 2 * jw - 1:2,
                           kx:kx + 2 * cfg.w3o - 1:2]

            def src_b3(j, jw, ky, kx):
                return x3b[0:65, 2 * j + ky:2 * j + ky + 2 * jw - 1:2,
                           kx:kx + 2 * cfg.w3o - 1:2]

            def emit3(j, jw, ci, ca, cb, ps):
                cw = cb - ca
                if ci == 0:
                    def wr(res, _j=j, _jw=jw):
                        cm_write(x4a[:, _j:_j + _jw, 2:2 + cfg.w3o], res)
                else:
                    def wr(res, _j=j, _jw=jw):
                        cm_write(x4b[0:64, _j:_j + _jw, 2:2 + cfg.w3o], res)
                requant_cm(ps[:cw, :jw, :], cw, m3sb[0:cw, ci:ci + 1],
                           scbc[0:cw, 8:9], scbc[0:cw, 5:6],
                           scbc[0:cw, 2:3], t3pool, "l3", [jw, cfg.w3o], wr)

            conv_ws(w3a, w3p, w3l, src_a3, src_t3, src_b3,
                    ((0, 128), (128, 192)), cfg.r3, cfg.w3o, RB3, ps3, emit3)

        # ============================ Layer 4 ============================
        with tc.tile_pool(name="l4ps", bufs=4, space="PSUM") as ps4, \
             tc.tile_pool(name="l4t", bufs=3) as t4pool:

            nc.sync.dma_start(out=t4t[0:64, :, :], in_=x4b[0:64, :, :])
            nc.sync.dma_start(out=t4t[64:128, 0:cfg.r3 - 1, :],
                              in_=x4b[0:64, 1:cfg.r3, :])

            def src_a4(j, jw, ky, kx):
                return x4a[0:128, 2 * j + ky:2 * j + ky + 2 * jw - 1:2,
                           kx:kx + 2 * cfg.w4o - 1:2]

            def src_t4(j, jw, ky, kx):
                return t4t[0:128, 2 * j + ky:2 * j + ky + 2 * jw - 1:2,
                           kx:kx + 2 * cfg.w4o - 1:2]

            def src_b4(j, jw, ky, kx):
                return x4b[0:65, 2 * j + ky:2 * j + ky + 2 * jw - 1:2,
                           kx:kx + 2 * cfg.w4o - 1:2]

            def emit4(j, jw, ci, ca, cb, ps):
                cw = cb - ca
                npix = jw * cfg.w4o
                s = t4pool.tile([128, cfg.r4, cfg.w4o], F32, tag="l4s")
                nc.scalar.activation(s[:cw, :jw, :], ps[:cw, :jw, :],
                                     mybir.ActivationFunctionType.Identity,
                                     bias=0.0, scale=m4sb[0:cw, ci:ci + 1])
                of = t4pool.tile([128, cfg.r4, cfg.w4o], F32, tag="l4o")
                if fast:
                    ti1 = t4pool.tile([128, cfg.r4, cfg.w4o], I32, tag="l4ti")
                    nc.gpsimd.tensor_copy(ti1[:cw, :jw, :], s[:cw, :jw, :])
                    nc.vector.tensor_copy(of[:cw, :jw, :], ti1[:cw, :jw, :])
                else:
                    u = t4pool.tile([128, cfg.r4, cfg.w4o], F32, tag="l4u")
                    nc.vector.tensor_scalar(u[:cw, :jw, :], s[:cw, :jw, :],
                                            0.5, None, AOP.add)
                    ti1 = t4pool.tile([128, cfg.r4, cfg.w4o], I32, tag="l4ti")
                    nc.gpsimd.tensor_copy(ti1[:cw, :jw, :], u[:cw, :jw, :])
                    g1 = t4pool.tile([128, cfg.r4, cfg.w4o], F32, tag="l4g")
                    nc.gpsimd.tensor_tensor(g1[:cw, :jw, :], ti1[:cw, :jw, :],
                                            u[:cw, :jw, :], AOP.is_gt)
                    nc.vector.tensor_tensor(of[:cw, :jw, :], ti1[:cw, :jw, :],
                                            g1[:cw, :jw, :], AOP.subtract)
                nc.scalar.dma_start(
                    out=bass.AP(out_h,
                                ca * (cfg.r4 * cfg.w4o) + j * cfg.w4o,
                                [[cfg.r4 * cfg.w4o, cw], [1, npix]]),
                    in_=of[:cw, :jw, :])

            conv_ws(w4a, w4p, w4l, src_a4, src_t4, src_b4,
                    ((0, 128), (128, 256), (256, 320)),
                    cfg.r4, cfg.w4o, cfg.r4, ps4, emit4)

        consts_cm.__exit__(None, None, None)

    nc.finalize()
    return nc


# ======================= host-side preparation =======================

def host_prep(inputs, cfg: Cfg):
    x = np.asarray(inputs["x"], np.float32)
    relus = np.asarray(inputs["relus"], np.float32)
    Bits = int(np.asarray(inputs["Bits"]))

    WX = cfg.W + 4
    W1P = cfg.w1o + 4
    W2P = cfg.w2o + 4
    W3P = cfg.w3o + 4

    def wprep(w, b, cout):
        wq = np.round(np.asarray(w, np.float32)).astype(np.float16)
        bq = np.round(np.asarray(b, np.float32)).astype(np.float16)
        wt = np.transpose(wq, (1, 2, 3, 0))  # [cin, ky, kx, cout]
        a = np.ascontiguousarray(wt[0:128].reshape(128, 25, cout))
        bm = np.zeros((65, 25, cout), np.float16)
        bm[0:64] = wt[128:192].reshape(64, 25, cout)
        bm[64, 24, :] = bq
        return a, bm

    w2a, w2b = wprep(inputs["w2"], inputs["b2"], 192)
    # pair weights: [tail of tap (2k,kx) ; tail of tap (2k+1,kx)]
    w2p = np.zeros((128, 10, 192), np.float16)
    for kyp in range(2):
        for kx in range(5):
            w2p[0:64, kyp * 5 + kx, :] = w2b[0:64, (2 * kyp) * 5 + kx, :]
            w2p[64:128, kyp * 5 + kx, :] = w2b[0:64, (2 * kyp + 1) * 5 + kx, :]
    # leftover ky=4 taps (bias/mask row lives on tap (4,4))
    w2l = np.zeros((65, 5, 192), np.float16)
    for kx in range(5):
        w2l[:, kx, :] = w2b[:, 20 + kx, :]
    w3a, w3b = wprep(inputs["w3"], inputs["b3"], 192)
    w4a, w4b = wprep(inputs["w4"], inputs["b4"], 320)

    def pairs_of(wb, cout):
        wp = np.zeros((128, 10, cout), np.float16)
        for kyp in range(2):
            for kx in range(5):
                wp[0:64, kyp * 5 + kx] = wb[0:64, (2 * kyp) * 5 + kx]
                wp[64:128, kyp * 5 + kx] = wb[0:64, (2 * kyp + 1) * 5 + kx]
        wl = np.zeros((65, 5, cout), np.float16)
        for kx in range(5):
            wl[:, kx] = wb[:, 20 + kx]
        return wp, wl

    w3p, w3l = pairs_of(w3b, 192)
    w4p, w4l = pairs_of(w4b, 320)

    wq1 = np.round(np.asarray(inputs["w1"], np.float32))
    bq1 = np.round(np.asarray(inputs["b1"], np.float32))
    w1m = np.zeros((76, 192), np.float16)
    kx_perm = [0, 2, 4, 1, 3]
    for ky in range(5):
        for c in range(3):
            for slot, kx in enumerate(kx_perm):
                w1m[ky * 15 + c * 5 + slot, :] = wq1[:, c, ky, kx]
    w1m[75, :] = bq1

    m1B = (np.asarray(inputs["muls0"], np.float32) *
           np.float32(2.0 ** (-(19 + IN_SCALE - CLP_K))))
    m2B = np.asarray(inputs["muls1"], np.float32) * np.float32(2.0 ** -16)
    m3B = np.asarray(inputs["muls2"], np.float32) * np.float32(2.0 ** -16)
    m4B = np.asarray(inputs["muls3"], np.float32) * np.float32(2.0 ** -15)
    m1 = np.zeros((128, 2), np.float32)
    m1[:, 0] = m1B[0:128]
    m1[0:64, 1] = m1B[128:192]
    m3c = np.zeros((128, 2), np.float32)
    m3c[:, 0] = m3B[0:128]
    m3c[0:64, 1] = m3B[128:192]
    m4c = np.zeros((128, 3), np.float32)
    m4c[:, 0] = m4B[0:128]
    m4c[:, 1] = m4B[128:256]
    m4c[0:64, 2] = m4B[256:320]

    clp = np.round((np.float32(2.0 ** Bits - 1) / relus) *
                   np.float32(2.0 ** (16 + CLP_K))).astype(np.float32)
    scl = np.floor((relus + np.float32(2.0 ** 3)) /
                   np.float32(2.0 ** 4)).astype(np.float32)
    Bl = np.array([2.0 ** -20, 2.0 ** -16, 2.0 ** -16], np.float32)
    sc = np.zeros(12, np.float32)
    sc[0:3] = clp[0:3] + np.float32(0.5)
    sc[3:6] = scl[0:3] * np.float32(2.0 ** -19)
    sc[6:9] = (clp[0:3] + np.float32(0.5)) * Bl

    in_maps = []
    for core in range(N_CORES):
        n, h = core // 2, core % 2
        a4 = cfg.r4 * h
        def maskvec(nslots, base, full):
            j = np.arange(nslots)
            return (((base + j) >= 0) & ((base + j) < full)).astype(np.float16)

        mk1 = maskvec(cfg.r1, 8 * a4 - 14, cfg.fr1)
        mk2 = maskvec(cfg.r2, 4 * a4 - 6, cfg.fr2)
        mk3 = maskvec(cfg.r3, 2 * a4 - 2, cfg.fr3)

        xpad = np.zeros((3, 2 * cfg.rx_half + 4, WX + 4), np.float32)
        t0 = 16 * a4 - 30
        lo = max(0, -t0)
        hi = min(cfg.rx, cfg.H - t0)
        if hi > lo:
            xpad[:, lo:hi, 2:2 + cfg.W] = x[n, :, t0 + lo:t0 + hi, :]
        # 76-plane im2col in R partition order: plane q=(ky,c,s):
        # xR[q][i,u] = xpad[c, 2i+ky, 2u+kx(s)]; plane 75 = rowmask/256
        WH = WX // 2
        nflat = 76 * cfg.rx_half
        npad = ((nflat + 127) // 128) * 128
        xs = np.zeros((npad, WH), np.float32)
        kx_of = [0, 2, 4, 1, 3]
        for ky in range(5):
            for cc in range(3):
                for s in range(5):
                    kx = kx_of[s]
                    q = ky * 15 + cc * 5 + s
                    pl = xpad[cc, ky:ky + 2 * cfg.rx_half:2,
                              kx:kx + 2 * WH:2]
                    xs[q * cfg.rx_half:(q + 1) * cfg.rx_half] = pl
        xs[75 * cfg.rx_half:75 * cfg.rx_half + cfg.r1] = \
            (mk1[:, None].astype(np.float32) / 256.0)

        mk4 = maskvec(cfg.r4, a4, cfg.fr4)

        def plane(nin, wid, mo):
            p = np.zeros((nin, wid), np.float16)
            for i in range(nin):
                if (i - 4) % 2 == 0:
                    j = (i - 4) // 2
                    if 0 <= j < len(mo):
                        p[i, :] = mo[j]
            return p

        mp2 = plane(cfg.r1, W1P, mk2)
        mp3 = plane(cfg.r2, W2P, mk3)
        mp4 = plane(cfg.r3, W3P, mk4)

        in_maps.append({
            "x": xs, "w1m": w1m, "w2a": w2a, "w2p": w2p, "w2l": w2l,
            "w3a": w3a,
            "w3p": w3p, "w3l": w3l, "w4a": w4a, "w4p": w4p,
            "w4l": w4l, "m1": m1, "m2": m2B,
            "m3": m3c, "m4": m4c, "sc": sc, "mp2": mp2,
            "mp3": mp3, "mp4": mp4,
        })
    return in_maps


def assemble_output(results, cfg: Cfg):
    out = np.empty((4, 320, cfg.fr4, cfg.w4o), np.float32)
    for core in range(N_CORES):
        n, h = core // 2, core % 2
        r = np.asarray(results[core]["out"])
        r = r.reshape(320, cfg.r4, cfg.w4o)
        out[n, :, cfg.r4 * h:cfg.r4 * (h + 1), :] = r
    return out


def fast_safe(inputs):
    """Interval-arithmetic proof that the 'fast' rounding chain is exact
    for this data (all rounding inputs bounded away from tie cases)."""
    relus = np.asarray(inputs["relus"], np.float64)
    if not np.all(np.isfinite(relus)) or np.any(relus <= 0):
        return False
    scl = np.floor((relus + 8.0) / 16.0)
    if np.any(scl < 0) or np.any(scl > 1):
        return False
    A = 255.0
    for wk, bk, mk, B in (("w1", "b1", "muls0", 2.0 ** -20),
                          ("w2", "b2", "muls1", 2.0 ** -16),
                          ("w3", "b3", "muls2", 2.0 ** -16),
                          ("w4", "b4", "muls3", 2.0 ** -15)):
        wq = np.round(np.asarray(inputs[wk], np.float64))
        bq = np.round(np.asarray(inputs[bk], np.float64))
        Q = (float(np.abs(wq).reshape(wq.shape[0], -1).sum(1).max()) * A +
             float(np.abs(bq).max()))
        if Q >= 2 ** 23:
            return False
        t = Q * float(np.abs(np.asarray(inputs[mk], np.float64)).max()) * B
        if not t < 0.45:
            return False
        A = 0.0  # this layer's outputs are provably exactly 0
    return True


# ======================================================================
# fp8 fast path
#
# When a host-side interval proof (fp8_gate) certifies that
#   - every requant stage's pre-floor value is < 0.44 in magnitude (so all
#     activations after L1's requant are exactly 0, clips never bind, and
#     round-half-even == the reference floor chain), and
#   - round(w2..4) are small ints exactly representable in fp8e4,
# the network runs as:
#   - quant: round/clip via the +1.5*2^23 magic-add trick (2 ops, exact),
#     on the compact parity image (no im2col duplication on-device);
#     im2col expansion happens as cheap DRAM->DRAM shift DMAs.
#   - L1: f16 weights-stationary conv (96-cout chunks), requant = 2 ops
#     (x*M+magic; -magic) writing fp8 straight into the SBUF-resident
#     [96, 2, rows, cols] activation tile (no DRAM round trip).
#   - L2: fp8 DoubleRow act-stationary conv (2 cin-planes per matmul),
#     requant 2 ops, PE transpose to channel-major fp8.
#   - L3/L4: fp8 DoubleRow weights-stationary conv, channel-major
#     requant (2 ops) directly into the next layer's input tile.
# Bias never enters the matmuls: its requant contribution is provably
# absorbed (|b*M| < 0.44 -> rounds to 0), which the gate checks.
# ======================================================================

F8 = mybir.dt.float8e4
DRMODE = mybir.MatmulPerfMode.DoubleRow
MAGIC = np.float32(12582912.0)  # 1.5 * 2^23
ACT_IDENT = mybir.ActivationFunctionType.Identity


class FCfg:
    r1, r2, r3, r4 = 149, 73, 35, 16
    w1o, w2o, w3o, w4o = 256, 128, 64, 32
    X2R, X2W = 149, 272      # row-interleaved planes; Ko step 272 %16==0
    X3R, X3W = 73, 144       # Ko step 144 %16==0
    X4R, X4W = 35, 80        # Ko step 80 %16==0
    PR = 312                 # xc rows per (c,par) plane
    RB1 = 38                 # L1 block rows
    PL = 152 * 256           # xqr plane elems (256-wide: packed rows)


def fp8_gate(inputs):
    """Interval proof that the fp8/magic-rounding program is bit-exact
    for these inputs (all post-L1 activations are exactly 0)."""
    try:
        x = np.asarray(inputs["x"], np.float64)
        relus = np.asarray(inputs["relus"], np.float64)
        Bits = int(np.asarray(inputs["Bits"]))
        split = int(np.asarray(inputs["split"]))
    except Exception:
        return False
    if x.shape != (4, 3, 512, 512):
        return False
    if not np.isfinite(x).all() or x.min() < 0:
        return False
    if not np.isfinite(relus).all() or (relus <= 0).any():
        return False
    if Bits < 0 or split < 1:
        return False
    import ml_dtypes
    A = 256.0  # fp8(clip(.,255)) can round up to 256
    for wk, bk, mk, B, wcap in (("w1", "b1", "muls0", 2.0 ** -20, 440),
                                ("w2", "b2", "muls1", 2.0 ** -16, 16),
                                ("w3", "b3", "muls2", 2.0 ** -16, 16),
                                ("w4", "b4", "muls3", 2.0 ** -15, 16)):
        w = np.asarray(inputs[wk], np.float64)
        b = np.asarray(inputs[bk], np.float64)
        m = np.asarray(inputs[mk], np.float64)
        if not (np.isfinite(w).all() and np.isfinite(b).all()
                and np.isfinite(m).all()):
            return False
        wq = np.round(w)
        bq = np.round(b)
        if np.abs(wq).max() > wcap or np.abs(bq).max() > 2048:
            return False
        # the kernel convolves with fp8-rounded weights; bound with those.
        # L1 weights are pre-scaled by 2^-5 so each fp8 product stays in
        # e4m3 range (|w*x| <= 448); the 2^5 is folded back into M1.
        scale = 2.0 ** -5 if wk == "w1" else 1.0
        w8 = (wq * scale).astype(np.float32).astype(ml_dtypes.float8_e4m3)
        w8 = w8.astype(np.float64) / scale
        if not np.isfinite(w8).all():
            return False
        if np.abs(w8 * scale).max() > 240.0:  # fp8e4(IEEE) max finite
            return False
        Q = (float(np.abs(w8).reshape(w8.shape[0], -1).sum(1).max()) * A +
             float(np.abs(bq).max()))
        if Q >= 2 ** 23:
            return False
        if not (Q * float(np.abs(m).max()) * B < 0.44):
            return False
        A = 0.0
    return True


def build_fast():
    c = FCfg
    nc = bacc.Bacc("TRN2", target_bir_lowering=False, debug=False,
                   num_devices=N_CORES, detect_race_conditions=True)

    xc_h = nc.declare_dram_parameter("xc", [128, 15 * 258], F32,
                                     isOutput=False)
    w1_h = nc.declare_dram_parameter("w1m", [38, 2, 192], F8, isOutput=False)
    w2_h = nc.declare_dram_parameter("w2d", [96, 25, 2, 192], F8,
                                     isOutput=False)
    w3_h = nc.declare_dram_parameter("w3d", [96, 25, 2, 192], F8,
                                     isOutput=False)
    w4_h = nc.declare_dram_parameter("w4d", [96, 25, 2, 320], F8,
                                     isOutput=False)
    mc_h = nc.declare_dram_parameter("mc", [128, 10], F32, isOutput=False)
    m2_h = nc.declare_dram_parameter("m2", [192], F32, isOutput=False)
    out_h = nc.declare_dram_parameter("out", [320, c.r4 * c.w4o], F32,
                                      isOutput=True)
    xqc_h = nc.dram_tensor("xqc", [1920, 258], F8)
    xqr_h = nc.dram_tensor("xqr", [75, 152, 256], F8)

    with tile.TileContext(nc) as tc:
        consts_cm = tc.tile_pool(name="consts", bufs=1)
        consts = consts_cm.__enter__()

        ident = consts.tile([128, 128], F16)
        make_identity(nc, ident)
        mc = consts.tile([128, 10], F32, tag="mc")
        nc.scalar.dma_start(out=mc, in_=mc_h[:])
        m2bc = consts.tile([128, 2, 192], F32, tag="m2bc")
        nc.scalar.dma_start(out=m2bc,
                          in_=bass.AP(m2_h, 0, [[0, 128], [0, 2], [1, 192]]))
        w1sb = consts.tile([38, 2, 192], F8, tag="w1sb")
        nc.scalar.dma_start(out=w1sb, in_=w1_h[:])
        w2sb = consts.tile([96, 25, 2, 192], F8, tag="w2sb")
        w3sb = consts.tile([96, 25, 2, 192], F8, tag="w3sb")
        w4sb = consts.tile([96, 25, 2, 320], F8, tag="w4sb")

        x2sb = consts.tile([96, c.X2R, 2, c.X2W], F8, tag="x2sb")
        x3sb = consts.tile([96, c.X3R, 2, c.X3W], F8, tag="x3sb")
        x4sb = consts.tile([96, c.X4R, 2, c.X4W], F8, tag="x4sb")
        nc.vector.memset(x2sb[:, :, :, 0:2], 0.0)
        nc.vector.memset(x2sb[:, :, :, 258:260], 0.0)
        nc.gpsimd.memset(x3sb[:, :, :, 0:2], 0.0)
        nc.gpsimd.memset(x3sb[:, :, :, 130:132], 0.0)
        nc.vector.memset(x4sb[:, :, :, 0:2], 0.0)
        nc.vector.memset(x4sb[:, :, :, 66:68], 0.0)

        MGC = mc[:, 7:8]
        NMGC = mc[:, 8:9]

        # ---------------- quant + im2col expansion ----------------
        FW = 15 * 258
        HW2 = FW // 2
        with tc.tile_pool(name="quant", bufs=1) as qp:
            xcin = qp.tile([128, FW], F32, tag="xcin")
            tq = qp.tile([128, FW], F32, tag="tq")
            xq16 = qp.tile([128, FW], F8, tag="xq16")
            QC = FW // 4
            for hf, (c0, c1) in enumerate(
                    ((0, QC), (QC, 2 * QC), (2 * QC, 3 * QC), (3 * QC, FW))):
                dq = (nc.sync, nc.scalar, nc.gpsimd, nc.sync)[hf]
                dq2 = (nc.sync, nc.scalar, nc.gpsimd, nc.scalar)[hf]
                dq.dma_start(out=xcin[:, c0:c1],
                             in_=bass.AP(xc_h, c0, [[FW, 128], [1, c1 - c0]]))
                # half-scale quant: fp8e4 (IEEE) max finite is 240, so
                # store round(x*128) <= 128 and fold the 2x into M1
                nc.vector.tensor_scalar(tq[:, c0:c1], xcin[:, c0:c1],
                                        128.0, MGC, AOP.mult, AOP.add)
                nc.gpsimd.tensor_scalar(xq16[:, c0:c1], tq[:, c0:c1],
                                        MGC, 128.0, AOP.subtract, AOP.min)
                dq2.dma_start(
                    out=bass.AP(xqc_h, c0, [[FW, 128], [1, c1 - c0]]),
                    in_=xq16[:, c0:c1])
            # expansion: per (slot-shift s, channel cc) D2D DMA writing the
            # 5 ky planes. dim0 = row (152) keeps the modeled cost low.
            ei = 0
            for s in range(5):
                par, sp = (0, s) if s < 3 else (1, s - 3)
                for cc in range(3):
                    eng = (nc.sync, nc.scalar, nc.gpsimd)[ei % 3]
                    ei += 1
                    eng.dma_start(
                        out=bass.AP(xqr_h, (cc * 5 + s) * c.PL,
                                    [[256, 152], [15 * c.PL, 5], [1, 256]]),
                        in_=bass.AP(
                            xqc_h,
                            (cc * 2 + par) * c.PR * 258 + sp,
                            [[516, 152], [258, 5], [1, 256]]))

        # engine rotation helpers for the 2-op requant
        ENGS = (nc.vector, nc.gpsimd, nc.scalar)

        def requant2(k, ps_ap, t_tile, t_sl, out_ap, Mcol, pw):
            # pass1 reads PSUM: DVE/Act only. pass2 is SBUF->SBUF: Pool ok.
            ea = (nc.vector, nc.scalar)[k % 2]
            eb = (nc.gpsimd, nc.gpsimd, nc.gpsimd, nc.scalar)[k % 4]
            if ea is nc.scalar:
                nc.scalar.activation(t_tile[t_sl], ps_ap, ACT_IDENT,
                                     bias=MGC[0:pw, :], scale=Mcol)
            else:
                ea.tensor_scalar(t_tile[t_sl], ps_ap, Mcol, MGC[0:pw, :],
                                 AOP.mult, AOP.add)
            if eb is nc.scalar:
                nc.scalar.activation(out_ap, t_tile[t_sl], ACT_IDENT,
                                     bias=NMGC[0:pw, :], scale=1.0)
            else:
                eb.tensor_scalar(out_ap, t_tile[t_sl], MGC[0:pw, :], None,
                                 AOP.subtract)

        for tp in range(0, 25, 5):
            nc.scalar.dma_start(out=w2sb[:, tp:tp + 5], in_=w2_h[:, tp:tp + 5])

        # ----------- Layers 1-4: dataflow-interleaved emission -----------
        # Program order follows data dependencies (L1 block -> the L2 row
        # groups it unblocks -> the L3 tiles those unblock) so the tile
        # scheduler keeps the PE fed from the start.
        l2t_cm = tc.tile_pool(name="l2t", bufs=4)
        l2t = l2t_cm.__enter__()
        l2ps_cm = tc.tile_pool(name="l2ps", bufs=4, space="PSUM")
        l2ps = l2ps_cm.__enter__()
        rkc = [0]
        l3p = {}

        def emit_l1_block(rpool, l1ps, l1t, blk):
            j0 = blk * c.RB1
            nj = min(c.RB1, c.r1 - j0)
            R = rpool.tile([38, 2, c.RB1, 256], F8, tag="R")
            rq = nc.sync if blk % 2 == 0 else nc.scalar
            rq2 = nc.scalar if blk % 2 == 0 else nc.sync
            splits = (0, 8, 20, nj) if blk == 0 else (0, nj)
            for pl in range(2):
                for si in range(len(splits) - 1):
                    r0, r1 = splits[si], splits[si + 1]
                    (rq if pl == 0 else rq2).dma_start(
                        out=R[:, pl, r0:r1, :],
                        in_=bass.AP(xqr_h,
                                    pl * 37 * c.PL + (j0 + r0) * 256,
                                    [[c.PL, 38], [256, r1 - r0], [1, 256]]))
            for g in range(0, nj, 4):
                jw = min(4, nj - g)
                for ci in range(2):
                    Mcol = mc[0:96, ci:ci + 1]
                    ps = l1ps.tile([96, 4, 256], F32, tag="ps")
                    for hh in range(0, jw, 2):
                        hw_ = min(2, jw - hh)
                        nc.tensor.matmul(
                            ps[:, hh:hh + hw_, :],
                            w1sb[:, :, ci * 96:ci * 96 + 96],
                            R[:, 0:2, g + hh:g + hh + hw_, 0:256],
                            start=True, stop=True, perf_mode=DRMODE)
                    t = l1t.tile([96, 4, 256], F32, tag="t")
                    requant2(rkc[0], ps[:, :jw, :], t,
                             (slice(0, 96), slice(0, jw)),
                             x2sb[0:96, j0 + g:j0 + g + jw, ci, 2:258],
                             Mcol, 96)
                    rkc[0] += 1

        def emit_l2_group(jg):
            nr = min(2, c.r2 - jg)
            ps2 = l2ps.tile([128, 2, 192], F32, tag="ps2")
            for r in range(nr):
                j = jg + r
                for ky in range(5):
                    for kx in range(5):
                        tap = ky * 5 + kx
                        nc.tensor.matmul(
                            ps2[:, r, :],
                            x2sb[0:96, 2 * j + ky, 0:2, kx:kx + 255:2],
                            w2sb[0:96, tap, 0:2, :],
                            start=(tap == 0), stop=(tap == 24),
                            perf_mode=DRMODE)
            u2 = l2t.tile([128, 2, 192], F32, tag="u2")
            nc.vector.tensor_tensor(u2[:, :nr, :], ps2[:, :nr, :],
                                    m2bc[:, :nr, :], AOP.mult)
            # qh2 slots hold cout 96-chunks padded to 128 cols so the xbar
            # DMA transpose (in free %128) can do the channel-major turn;
            # pad cols transpose into partitions 96..127, never read.
            qh2 = l2t.tile([128, 2, 2, 128], F16, tag="qh2")
            for i in range(2):
                nc.gpsimd.tensor_scalar(qh2[:, :nr, i, 0:96],
                                        u2[:, :nr, i * 96:i * 96 + 96],
                                        MGC, MGC, AOP.add, AOP.subtract)
            xt = l2t.tile([128, 2, 2, 128], F16, tag="xt")
            for r in range(nr):
                for i in range(2):
                    eng = nc.scalar if (jg // 2 + r + i) % 2 == 0 else nc.sync
                    eng.dma_start(out=xt[:, r, i, :], in_=qh2[:, r, i, :],
                                  transpose=True)
            for i in range(2):
                nc.gpsimd.tensor_copy(x3sb[0:96, jg:jg + nr, i, 2:130],
                                      xt[0:96, 0:nr, i, 0:128])

        def emit_l3_tile(j0):
            l3ps, l3t = l3p["ps"], l3p["t"]
            jw = min(8, c.r3 - j0)
            for ci in range(2):
                ps3 = l3ps.tile([96, 8, 64], F32, tag="ps3")
                for ky in range(5):
                    for kx in range(5):
                        tap = ky * 5 + kx
                        nc.tensor.matmul(
                            ps3[:, :jw, :],
                            w3sb[0:96, tap, 0:2, ci * 96:ci * 96 + 96],
                            x3sb[0:96,
                                 2 * j0 + ky:2 * j0 + ky + 2 * jw - 1:2,
                                 0:2, kx:kx + 127:2].rearrange(
                                     "k r t c -> k t r c"),
                            start=(tap == 0), stop=(tap == 24),
                            perf_mode=DRMODE)
                t3 = l3t.tile([96, 8, 64], F32, tag="t3")
                requant2(rkc[0], ps3[:, :jw, :], t3,
                         (slice(0, 96), slice(0, jw)),
                         x4sb[0:96, j0:j0 + jw, ci, 2:66],
                         mc[0:96, 2 + ci:3 + ci], 96)
                rkc[0] += 1

        def emit_l4_half(l4ps, l4t, j0h, jh):
            for ci, (ca, cb) in enumerate(((0, 128), (128, 256), (256, 320))):
                cw = cb - ca
                ps4 = l4ps.tile([128, 8, 32], F32, tag="ps4")
                for ky in range(5):
                    for kx in range(5):
                        tap = ky * 5 + kx
                        nc.tensor.matmul(
                            ps4[:cw, :jh, :],
                            w4sb[0:96, tap, 0:2, ca:cb],
                            x4sb[0:96,
                                 2 * j0h + ky:2 * j0h + ky + 2 * jh - 1:2,
                                 0:2, kx:kx + 63:2].rearrange(
                                     "k r t c -> k t r c"),
                            start=(tap == 0), stop=(tap == 24),
                            perf_mode=DRMODE)
                t4 = l4t.tile([128, 8, 32], F32, tag="t4")
                ea = (nc.vector, nc.scalar, nc.vector)[ci]
                if ea is nc.scalar:
                    nc.scalar.activation(t4[:cw, :jh], ps4[:cw, :jh],
                                         ACT_IDENT, bias=MGC[0:cw, :],
                                         scale=mc[0:cw, 4 + ci:5 + ci])
                else:
                    ea.tensor_scalar(t4[:cw, :jh], ps4[:cw, :jh],
                                     mc[0:cw, 4 + ci:5 + ci], MGC[0:cw, :],
                                     AOP.mult, AOP.add)
                of = l4t.tile([128, 8, 32], F32, tag="of")
                nc.gpsimd.tensor_scalar(of[:cw, :jh], t4[:cw, :jh],
                                        MGC[0:cw, :], None, AOP.subtract)
                nc.sync.dma_start(
                    out=bass.AP(out_h, ca * 512 + j0h * 32,
                                [[512, cw], [1, jh * 32]]),
                    in_=of[:cw, :jh])

        with tc.tile_pool(name="l1r", bufs=2) as rpool, \
             tc.tile_pool(name="l1ps", bufs=2, space="PSUM") as l1ps, \
             tc.tile_pool(name="l1t", bufs=3) as l1t:
            for blk in range(4):
                if blk == 2:
                    # L3/L4 weights arrive while L1 runs (sync is quiet);
                    # chunked so hoisted pieces can't block the queue long
                    for tp in range(0, 25, 5):
                        nc.sync.dma_start(out=w3sb[:, tp:tp + 5],
                                          in_=w3_h[:, tp:tp + 5])
                        nc.sync.dma_start(out=w4sb[:, tp:tp + 5],
                                          in_=w4_h[:, tp:tp + 5])
                emit_l1_block(rpool, l1ps, l1t, blk)

        for jg in range(0, 38, 2):
            emit_l2_group(jg)

        with tc.tile_pool(name="l3ps", bufs=2, space="PSUM") as l3ps, \
             tc.tile_pool(name="l3t", bufs=2) as l3t:
            l3p["ps"], l3p["t"] = l3ps, l3t
            emit_l3_tile(0)                     # needs x3sb rows <= 18
            for jg in range(38, 54, 2):
                emit_l2_group(jg)
            emit_l3_tile(8)                     # rows <= 34
            emit_l3_tile(16)                    # rows <= 50
            with tc.tile_pool(name="l4ps", bufs=2, space="PSUM") as l4ps, \
                 tc.tile_pool(name="l4t", bufs=3) as l4t:
                emit_l4_half(l4ps, l4t, 0, 8)   # x4sb rows <= 18
                for jg in range(54, c.r2, 2):
                    emit_l2_group(jg)
                emit_l3_tile(24)
                emit_l3_tile(32)
                emit_l4_half(l4ps, l4t, 8, 8)

        l2ps_cm.__exit__(None, None, None)
        l2t_cm.__exit__(None, None, None)

        consts_cm.__exit__(None, None, None)

    nc.finalize()
    return nc


def host_prep_fast(inputs):
    import ml_dtypes
    c = FCfg
    F8NP = ml_dtypes.float8_e4m3
    x = np.asarray(inputs["x"], np.float32)

    wq1 = np.round(np.asarray(inputs["w1"], np.float32))
    w1f = np.zeros((76, 192), np.float32)
    for ky in range(5):
        for cc in range(3):
            for s in range(5):
                kx = 2 * s if s < 3 else 2 * (s - 3) + 1
                w1f[ky * 15 + cc * 5 + s, :] = wq1[:, cc, ky, kx]
    w1p = np.zeros((38, 2, 192), np.float32)
    w1p[:, 0, :] = w1f[0:38]
    w1p[1:38, 1, :] = w1f[38:75]   # w1p[0,1] stays 0: plane 37 dup guard
    w1m = (w1p * np.float32(2.0 ** -5)).astype(F8NP)

    def wdr(wk, cout):
        wq = np.round(np.asarray(inputs[wk], np.float32))
        # [cout, 192, 5, 5] -> [96, 25, 2, cout]
        arr = np.transpose(wq.reshape(cout, 2, 96, 25), (2, 3, 1, 0))
        return np.ascontiguousarray(arr).astype(F8NP)

    w2d = wdr("w2", 192)
    w3d = wdr("w3", 192)
    w4d = wdr("w4", 320)

    m1B = np.asarray(inputs["muls0"], np.float32) * np.float32(2.0 ** -14)
    m2B = np.asarray(inputs["muls1"], np.float32) * np.float32(2.0 ** -16)
    m3B = np.asarray(inputs["muls2"], np.float32) * np.float32(2.0 ** -16)
    m4B = np.asarray(inputs["muls3"], np.float32) * np.float32(2.0 ** -15)
    mc = np.zeros((128, 10), np.float32)
    mc[0:96, 0] = m1B[0:96]
    mc[0:96, 1] = m1B[96:192]
    mc[0:96, 2] = m3B[0:96]
    mc[0:96, 3] = m3B[96:192]
    mc[:, 4] = m4B[0:128]
    mc[:, 5] = m4B[128:256]
    mc[0:64, 6] = m4B[256:320]
    mc[:, 7] = MAGIC
    mc[:, 8] = -MAGIC

    in_maps = []
    for core in range(N_CORES):
        n, h = core // 2, core % 2
        a4 = c.r4 * h
        xpad = np.zeros((3, 307, 516), np.float32)
        t0 = 16 * a4 - 30
        lo = max(0, -t0)
        hi = min(307, 512 - t0)
        if hi > lo:
            xpad[:, lo:hi, 2:514] = x[n, :, t0 + lo:t0 + hi, :]
        xp = np.zeros((3, 2, c.PR, 258), np.float32)
        xp[:, 0, 0:307, :] = xpad[:, :, 0::2]
        xp[:, 1, 0:307, :] = xpad[:, :, 1::2]
        flat = np.zeros((1920, 258), np.float32)
        flat[0:1872] = xp.reshape(1872, 258)
        in_maps.append({
            "xc": np.ascontiguousarray(flat.reshape(128, 15 * 258)),
            "w1m": w1m, "w2d": w2d, "w3d": w3d, "w4d": w4d,
            "mc": mc, "m2": m2B,
        })
    return in_maps


def assemble_fast(results):
    c = FCfg
    out = np.empty((4, 320, 32, 32), np.float32)
    for core in range(N_CORES):
        n, h = core // 2, core % 2
        r = np.asarray(results[core]["out"]).reshape(320, c.r4, c.w4o)
        out[n, :, c.r4 * h:c.r4 * (h + 1), :] = r
    return out


_cached = {}


def _get_nc(fast):
    key = "fast" if fast else "exact"
    if key not in _cached:
        _cached[key] = build_program(Cfg(H=512, W=512, rows4=16), fast=fast)
    return _cached[key]


def _get_nc_fp8():
    if "fp8" not in _cached:
        _cached["fp8"] = build_fast()
    return _cached["fp8"]


def kernel(**inputs) -> np.ndarray:
    if fp8_gate(inputs):
        nc = _get_nc_fp8()
        in_maps = host_prep_fast(inputs)
        res = run_bass_kernel_spmd(nc, in_maps, core_ids=list(range(N_CORES)))
        return assemble_fast(res.results)
    cfg = Cfg(H=512, W=512, rows4=16)
    nc = _get_nc(fast_safe(inputs))
    in_maps = host_prep(inputs, cfg)
    res = run_bass_kernel_spmd(nc, in_maps, core_ids=list(range(N_CORES)))
    return assemble_output(res.results, cfg)


def run_traced(**inputs):
    if fp8_gate(inputs):
        nc = build_fast()
        in_maps = host_prep_fast(inputs)
        res = run_bass_kernel_spmd(nc, in_maps,
                                   core_ids=list(range(N_CORES)), trace=True)
        return assemble_fast(res.results), res
    cfg = Cfg(H=512, W=512, rows4=16)
    nc = build_program(cfg, fast=fast_safe(inputs))
    in_maps = host_prep(inputs, cfg)
    res = run_bass_kernel_spmd(nc, in_maps, core_ids=list(range(N_CORES)),
                               trace=True)
    return assemble_output(res.results, cfg), res



# revision 49
# speedup vs baseline: 1.0057x; 1.0057x over previous
"""Trainium2 Bass kernel for the 4-layer quantized strided CNN.

Strategy:
  - Pure data parallelism: 8 cores = 4 batch x 2 H-halves. One uniform SPMD
    program; per-core differences enter only through input data (shards +
    mask planes).
  - Forward-pass identity: sum_i floor((round(w)+i)/s) == round(w), so the
    split-loop qconv collapses to ONE conv with integer weights round(w) and
    bias round(b). All arithmetic on integers < 2^24 is exact in f32/fp16.
  - Activations/weights stored fp16 (integers up to 2048 exact), matmul on
    the PE at 1 cycle/row with fp32 PSUM accumulation -> bit-exact vs the
    f32 jax reference.
  - L1 (Cin=3): weights-stationary, im2col activations (built via DMA
    gathers from a parity-split DRAM copy of the quantized input).
  - L2..L4 (Cin=192): activations-stationary [K=cin-chunk, M=128 pixels],
    moving weights [K, Cout]; conv = 25 taps x 2 K-chunks accumulated in
    PSUM. Bias is folded into the matmul via a ones/mask row appended to the
    K=64 tail chunk; the mask row also zeroes out-of-image rows so they
    requantize to exactly 0.
  - Requant chain is bit-exact: floor(v) = i32cast(v) - (i32cast(v) > v)
    (HW f32->i32 cast is round-half-even, verified), pow2 scalings commute
    with fp32 rounding.
  - PE transposes (via identity) convert [pix, cout] back to channel-major
    for the next layer. L4 output stays [pix, cout]; host transposes.
"""

import numpy as np

import concourse.bass as bass
import concourse.bacc as bacc
import concourse.mybir as mybir
import concourse.tile as tile
from concourse.bass_utils import run_bass_kernel_spmd
from concourse.masks import make_identity

F32 = mybir.dt.float32
F16 = mybir.dt.float16
I32 = mybir.dt.int32
AOP = mybir.AluOpType

N_CORES = 8
CLP_K = 7
IN_SCALE = 8


class Cfg:
    """Geometry for the uniform per-core program."""

    def __init__(self, H=512, W=512, rows4=16):
        self.H, self.W = H, W
        self.r4 = rows4                    # L4 out rows per core
        self.r3 = 2 * rows4 + 3            # L3 out slots (window)
        self.r2 = 4 * rows4 + 9            # L2 out slots
        self.r1 = 8 * rows4 + 21           # L1 out slots
        self.rx = 16 * rows4 + 45          # x rows per shard
        self.w1o = W // 2
        self.w2o = W // 4
        self.w3o = W // 8
        self.w4o = W // 16
        self.fr1, self.fr2 = H // 2, H // 4
        self.fr3, self.fr4 = H // 8, H // 16
        self.rx_half = (self.rx + 1) // 2


def build_program(cfg: Cfg, detect_races=True, fast=True):
    nc = bacc.Bacc("TRN2", target_bir_lowering=False, debug=False,
                   num_devices=N_CORES,
                   detect_race_conditions=detect_races)

    WX = cfg.W + 4
    W1P = cfg.w1o + 4
    W2P = cfg.w2o + 4
    W3P = cfg.w3o + 4

    # ---------------- parameters ----------------
    x_h = None  # declared below once nrows_pad is known
    w1_h = nc.declare_dram_parameter("w1m", [76, 192], F16, isOutput=False)
    w2a_h = nc.declare_dram_parameter("w2a", [128, 25, 192], F16, isOutput=False)
    w2p_h = nc.declare_dram_parameter("w2p", [128, 10, 192], F16, isOutput=False)
    w2l_h = nc.declare_dram_parameter("w2l", [65, 5, 192], F16, isOutput=False)
    w3a_h = nc.declare_dram_parameter("w3a", [128, 25, 192], F16, isOutput=False)
    w3p_h = nc.declare_dram_parameter("w3p", [128, 10, 192], F16, isOutput=False)
    w3l_h = nc.declare_dram_parameter("w3l", [65, 5, 192], F16, isOutput=False)
    w4a_h = nc.declare_dram_parameter("w4a", [128, 25, 320], F16, isOutput=False)
    w4p_h = nc.declare_dram_parameter("w4p", [128, 10, 320], F16, isOutput=False)
    w4l_h = nc.declare_dram_parameter("w4l", [65, 5, 320], F16, isOutput=False)
    m1_h = nc.declare_dram_parameter("m1", [128, 2], F32, isOutput=False)
    m2_h = nc.declare_dram_parameter("m2", [192], F32, isOutput=False)
    m3_h = nc.declare_dram_parameter("m3", [128, 2], F32, isOutput=False)
    m4_h = nc.declare_dram_parameter("m4", [128, 3], F32, isOutput=False)
    sc_h = nc.declare_dram_parameter("sc", [12], F32, isOutput=False)
    mp2_h = nc.declare_dram_parameter("mp2", [cfg.r1, W1P], F16, isOutput=False)
    mp3_h = nc.declare_dram_parameter("mp3", [cfg.r2, W2P], F16, isOutput=False)
    mp4_h = nc.declare_dram_parameter("mp4", [cfg.r3, W3P], F16, isOutput=False)
    out_h = nc.declare_dram_parameter("out", [320, cfg.r4 * cfg.w4o], F32,
                                      isOutput=True)

    x_h = nc.declare_dram_parameter(
        "x", [((76 * cfg.rx_half + 127) // 128) * 128, WX // 2], F32,
        isOutput=False)
    xq_h = nc.dram_tensor(
        "xq_par", [((76 * cfg.rx_half + 127) // 128) * 128, WX // 2], F16)
    RB1 = 38
    x2_bounds = list(range(0, cfg.r1, RB1)) + [cfg.r1]
    x2s_h = [nc.dram_tensor(f"x2s{k}",
                            [193, x2_bounds[k + 1] - x2_bounds[k], W1P], F16)
             for k in range(len(x2_bounds) - 1)]

    nrows_flat = 76 * cfg.rx_half
    rows_pp = (nrows_flat + 127) // 128       # flat rows per partition
    nrows_pad = rows_pp * 128

    with tile.TileContext(nc) as tc:
        consts_cm = tc.tile_pool(name="consts", bufs=1)
        consts = consts_cm.__enter__()

        ident = consts.tile([128, 128], F16)
        make_identity(nc, ident)

        def load(h, shape, dt=F16, tag=None):
            t = consts.tile(shape, dt, tag=tag)
            nc.sync.dma_start(out=t, in_=h[:])
            return t

        w1sb = load(w1_h, [76, 192], tag="w1sb")
        t3t = consts.tile([128, cfg.r2, W2P], F16, tag="t3t")
        t4t = consts.tile([128, cfg.r3, W3P], F16, tag="t4t")
        m1sb = load(m1_h, [128, 2], F32, tag="m1sb")

        def bcast_tile(src_h, n, tag):
            t = consts.tile([128, n], F32, tag=tag)
            nc.sync.dma_start(out=t, in_=bass.AP(src_h, 0, [[0, 128], [1, n]]))
            return t

        scbc = bcast_tile(sc_h, 12, "scbc")
        half_col = consts.tile([128, 1], F32)
        nc.vector.memset(half_col, 0.5)

        x3a = consts.tile([128, cfg.r2, W2P], F16)
        x3b = consts.tile([65, cfg.r2, W2P], F16)
        x4a = consts.tile([128, cfg.r3, W3P], F16)
        x4b = consts.tile([65, cfg.r3, W3P], F16)
        for t_, wp in ((x3a, W2P), (x3b, W2P), (x4a, W3P), (x4b, W3P)):
            nc.vector.memset(t_[:, :, 0:2], 0.0)
            nc.vector.memset(t_[:, :, wp - 2:wp], 0.0)
        nc.sync.dma_start(out=x3b[64:65, :, :], in_=mp3_h[:])
        nc.sync.dma_start(out=x4b[64:65, :, :], in_=mp4_h[:])

        # =========== input quantization: xq = clip(rhe(x*256),0,255) ===========
        # x arrives host-parity-split + padded; quant = one elementwise pass.
        # Partition p holds flat rows [p*rows_pp, (p+1)*rows_pp).
        WH = WX // 2
        fpp = rows_pp * WH
        NQC = max(1, (fpp * 20 + 84999) // 85000)  # chunk to fit SBUF
        qc = (fpp + NQC - 1) // NQC
        with tc.tile_pool(name="quant", bufs=2) as qpool:
            for ci_ in range(NQC):
                f0 = ci_ * qc
                fw = min(qc, fpp - f0)
                eng_in = nc.sync if ci_ % 2 == 0 else nc.scalar
                eng_out = nc.scalar if ci_ % 2 == 0 else nc.sync
                xin = qpool.tile([128, qc], F32, tag="xin")
                eng_in.dma_start(
                    out=xin[:, :fw],
                    in_=bass.AP(x_h, f0, [[fpp, 128], [1, fw]]))
                ti = qpool.tile([128, qc], I32, tag="ti")
                nc.vector.tensor_scalar(ti[:, :fw], xin[:, :fw], 256.0, None,
                                        AOP.mult)
                xqt = qpool.tile([128, qc], F16, tag="xqt")
                nc.gpsimd.tensor_scalar(xqt[:, :fw], ti[:, :fw], 0.0, 255.0,
                                        AOP.max, AOP.min)
                eng_out.dma_start(
                    out=bass.AP(xq_h, f0, [[fpp, 128], [1, fw]]),
                    in_=xqt[:, :fw])
        # x2 mask plane 192 <- mp2 (per split tensor)
        for k in range(len(x2s_h)):
            b0, b1 = x2_bounds[k], x2_bounds[k + 1]
            nc.scalar.dma_start(
                out=bass.AP(x2s_h[k], 192 * (b1 - b0) * W1P,
                            [[W1P, b1 - b0], [1, W1P]]),
                in_=bass.AP(mp2_h, b0 * W1P, [[W1P, b1 - b0], [1, W1P]]))

        # ============================ Layer 1 ============================
        STG = 8
        with tc.tile_pool(name="l1R", bufs=2) as rpool, \
             tc.tile_pool(name="l1ps", bufs=3, space="PSUM") as pspool, \
             tc.tile_pool(name="l1t", bufs=2) as tpool, \
             tc.tile_pool(name="l1s", bufs=4) as spool:

            def _l1_pair(ci, ca, cb, cw, R, j, jw, st, sr):
                ps = pspool.tile([128, 4, cfg.w1o], F32, tag="ps")
                for mj in range(0, jw, 2):
                    mw = min(2, jw - mj)
                    nc.tensor.matmul(
                        ps[:cw, mj:mj + mw, :], w1sb[:, ca:cb],
                        R[:, j + mj:j + mj + mw, 0:cfg.w1o],
                        start=True, stop=True)
                s = tpool.tile([128, 4, cfg.w1o], F32, tag="s")
                nc.scalar.activation(
                    s[:cw, :jw, :], ps[:cw, :jw, :],
                    mybir.ActivationFunctionType.Identity,
                    bias=half_col[0:cw, :], scale=m1sb[0:cw, ci:ci + 1])
                dst = st[:cw, sr:sr + jw, 2:2 + cfg.w1o]
                if fast:
                    v = tpool.tile([128, 4, cfg.w1o], F32, tag="v")
                    nc.vector.tensor_scalar(
                        v[:cw, :jw, :], s[:cw, :jw, :],
                        scbc[0:cw, 0:1], scbc[0:cw, 3:4], AOP.min, AOP.mult)
                    ti2 = tpool.tile([128, 4, cfg.w1o], I32, tag="ti2")
                    nc.gpsimd.tensor_copy(ti2[:cw, :jw, :], v[:cw, :jw, :])
                    if (j // 4) % 3 == 0:
                        nc.gpsimd.tensor_copy(dst, ti2[:cw, :jw, :])
                    else:
                        nc.vector.tensor_copy(dst, ti2[:cw, :jw, :])
                    return
                s2 = tpool.tile([128, 4, cfg.w1o], F32, tag="s2")
                nc.vector.tensor_scalar(
                    s2[:cw, :jw, :], s[:cw, :jw, :],
                    0.0, scbc[0:cw, 0:1], AOP.max, AOP.min)
                ti1 = tpool.tile([128, 4, cfg.w1o], I32, tag="ti1")
                nc.gpsimd.tensor_copy(ti1[:cw, :jw, :], s2[:cw, :jw, :])
                g1 = tpool.tile([128, 4, cfg.w1o], F32, tag="g1")
                nc.gpsimd.tensor_tensor(
                    g1[:cw, :jw, :], ti1[:cw, :jw, :], s2[:cw, :jw, :],
                    AOP.is_gt)
                c1t = tpool.tile([128, 4, cfg.w1o], F32, tag="c1t")
                nc.vector.tensor_tensor(
                    c1t[:cw, :jw, :], ti1[:cw, :jw, :], g1[:cw, :jw, :],
                    AOP.subtract)
                v = tpool.tile([128, 4, cfg.w1o], F32, tag="v")
                nc.vector.tensor_scalar(
                    v[:cw, :jw, :], c1t[:cw, :jw, :],
                    scbc[0:cw, 3:4], 0.5, AOP.mult, AOP.add)
                ti2 = tpool.tile([128, 4, cfg.w1o], I32, tag="ti2")
                nc.gpsimd.tensor_copy(ti2[:cw, :jw, :], v[:cw, :jw, :])
                g2 = tpool.tile([128, 4, cfg.w1o], F32, tag="g2")
                nc.gpsimd.tensor_tensor(
                    g2[:cw, :jw, :], ti2[:cw, :jw, :], v[:cw, :jw, :],
                    AOP.is_gt)
                nc.vector.tensor_tensor(
                    dst, ti2[:cw, :jw, :], g2[:cw, :jw, :], AOP.subtract)

            wload = {}
            n_blk = (cfg.r1 + RB1 - 1) // RB1
            for blk in range(n_blk):
                j0 = blk * RB1
                nj = min(RB1, cfg.r1 - j0)
                R = rpool.tile([76, RB1, WX // 2], F16, tag="R")
                nc.sync.dma_start(
                    out=R[:, :nj, :],
                    in_=bass.AP(xq_h, j0 * WH,
                                [[cfg.rx_half * WH, 76], [1, nj * WH]]))
                if blk == 0:
                    wload[0] = (load(w2a_h, [128, 25, 192], tag="w2a"),
                                load(w2p_h, [128, 10, 192], tag="w2p"),
                                load(w2l_h, [65, 5, 192], tag="w2l"),
                                bcast_tile(m2_h, 192, "m2bc"))
                elif blk == 1:
                    wload[1] = (load(w3a_h, [128, 25, 192], tag="w3a"),
                                load(w3p_h, [128, 10, 192], tag="w3p"),
                                load(w3l_h, [65, 5, 192], tag="w3l"),
                                load(w4a_h, [128, 25, 320], tag="w4a"),
                                load(w4p_h, [128, 10, 320], tag="w4p"),
                                load(w4l_h, [65, 5, 320], tag="w4l"),
                                load(m3_h, [128, 2], F32, tag="m3sb"),
                                load(m4_h, [128, 3], F32, tag="m4sb"))

                for ci, (ca, cb) in enumerate(((0, 128), (128, 192))):
                    cw = cb - ca
                    for g0 in range(0, nj, STG):
                        gw = min(STG, nj - g0)
                        st = spool.tile([128, STG, W1P], F16, tag="st")
                        nc.vector.memset(st[:cw, :gw, 0:2], 0.0)
                        nc.vector.memset(st[:cw, :gw, W1P - 2:W1P], 0.0)
                        for j in range(g0, g0 + gw, 4):
                            jw = min(4, g0 + gw - j)
                            _l1_pair(ci, ca, cb, cw, R, j, jw, st, j - g0)
                        rk_ = x2_bounds[blk + 1] - x2_bounds[blk]
                        nc.scalar.dma_start(
                            out=bass.AP(x2s_h[blk],
                                        (ca * rk_ + (j0 + g0 -
                                                     x2_bounds[blk])) * W1P,
                                        [[rk_ * W1P, cw], [W1P, gw],
                                         [1, W1P]]),
                            in_=st[:cw, :gw, :])

        # =================== requant for [pix, cout] layout ===================
        def requant_full(q_ps, pw, cout, mbc, clp_col, scl_col, c5s_col,
                         tpool, tag):
            t1 = tpool.tile([128, cout], F32, tag=tag + "t1")
            nc.vector.tensor_tensor(t1[:pw], q_ps[:pw], mbc[:pw], AOP.mult)
            qf = tpool.tile([128, cout], F16, tag=tag + "qf")
            if fast:
                s = tpool.tile([128, cout], F32, tag=tag + "s")
                nc.scalar.activation(s[:pw], t1[:pw],
                                     mybir.ActivationFunctionType.Identity,
                                     bias=half_col[:pw, :], scale=1.0)
                v = tpool.tile([128, cout], F32, tag=tag + "v")
                nc.vector.tensor_scalar(v[:pw], s[:pw], clp_col[:pw],
                                        scl_col[:pw], AOP.min, AOP.mult)
                ti2 = tpool.tile([128, cout], I32, tag=tag + "ti2")
                nc.gpsimd.tensor_copy(ti2[:pw], v[:pw])
                nc.gpsimd.tensor_copy(qf[:pw], ti2[:pw])
                return qf
            s = tpool.tile([128, cout], F32, tag=tag + "s")
            nc.vector.tensor_scalar(s[:pw], t1[:pw], 0.5, 0.0,
                                    AOP.add, AOP.max)
            s2 = tpool.tile([128, cout], F32, tag=tag + "s2")
            nc.vector.tensor_scalar(s2[:pw], s[:pw], clp_col[:pw], None,
                                    AOP.min)
            ti1 = tpool.tile([128, cout], I32, tag=tag + "ti1")
            nc.gpsimd.tensor_copy(ti1[:pw], s2[:pw])
            g1 = tpool.tile([128, cout], F32, tag=tag + "g1")
            nc.gpsimd.tensor_tensor(g1[:pw], ti1[:pw], s2[:pw], AOP.is_gt)
            c1 = tpool.tile([128, cout], F32, tag=tag + "c1")
            nc.gpsimd.tensor_tensor(c1[:pw], ti1[:pw], g1[:pw], AOP.subtract)
            v = tpool.tile([128, cout], F32, tag=tag + "v")
            nc.vector.tensor_scalar(v[:pw], c1[:pw], scl_col[:pw], 0.5,
                                    AOP.mult, AOP.add)
            ti2 = tpool.tile([128, cout], I32, tag=tag + "ti2")
            nc.gpsimd.tensor_copy(ti2[:pw], v[:pw])
            g2 = tpool.tile([128, cout], F32, tag=tag + "g2")
            nc.gpsimd.tensor_tensor(g2[:pw], ti2[:pw], v[:pw], AOP.is_gt)
            nc.vector.tensor_tensor(qf[:pw], ti2[:pw], g2[:pw], AOP.subtract)
            return qf

        def conv_tiles(src_a, src_b, wa, wb, cout, n_out_rows, out_w,
                       rows_per_tile, pspool, emit_out):
            j = 0
            while j < n_out_rows:
                jw = min(rows_per_tile, n_out_rows - j)
                pw = jw * out_w
                ps = pspool.tile([128, cout], F32, tag="cps")
                first = True
                for ky in range(5):
                    for kx in range(5):
                        tap = ky * 5 + kx
                        last = (ky == 4 and kx == 4)
                        nc.tensor.matmul(ps[:pw], src_a(j, jw, ky, kx),
                                         wa[:, tap, :], start=first,
                                         stop=False)
                        first = False
                        nc.tensor.matmul(ps[:pw], src_b(j, jw, ky, kx),
                                         wb[:, tap, :], start=False, stop=last)
                emit_out(j, jw, ps, pw)
                j += jw

        w2a, w2p, w2l, m2bc = wload[0]
        if 1 not in wload:
            wload[1] = (load(w3a_h, [128, 25, 192], tag="w3a"),
                        load(w3p_h, [128, 10, 192], tag="w3p"),
                        load(w3l_h, [65, 5, 192], tag="w3l"),
                        load(w4a_h, [128, 25, 320], tag="w4a"),
                        load(w4p_h, [128, 10, 320], tag="w4p"),
                        load(w4l_h, [65, 5, 320], tag="w4l"),
                        load(m3_h, [128, 2], F32, tag="m3sb"),
                        load(m4_h, [128, 3], F32, tag="m4sb"))
        w3a, w3p, w3l, w4a, w4p, w4l, m3sb, m4sb = wload[1]

        # ============================ Layer 2 ============================
        # Tail-chunk pairing: cin 128..191 of taps (ky,ky+1) packed into one
        # K=128 contraction via a row-shifted tail tile T. 25 full + 10 pair
        # + 5 leftover = 40 matmuls/tile (vs 50).
        RB2 = 10
        with tc.tile_pool(name="l2r", bufs=2) as r2pool, \
             tc.tile_pool(name="l2ps", bufs=4, space="PSUM") as ps2, \
             tc.tile_pool(name="l2tr", bufs=2, space="PSUM") as tr2, \
             tc.tile_pool(name="l2t", bufs=2) as t2pool:
            n_blk = (cfg.r2 + RB2 - 1) // RB2
            for blk in range(n_blk):
                j0 = blk * RB2
                nj = min(RB2, cfg.r2 - j0)
                nin = 2 * nj + 3
                def x2_read(dst, d0, np_, pl0, gr0, nrows):
                    for k in range(len(x2s_h)):
                        b0, b1 = x2_bounds[k], x2_bounds[k + 1]
                        lo, hi = max(gr0, b0), min(gr0 + nrows, b1)
                        if hi > lo:
                            rk = b1 - b0
                            nc.sync.dma_start(
                                out=dst[d0:d0 + np_,
                                        lo - gr0:hi - gr0, :],
                                in_=bass.AP(
                                    x2s_h[k],
                                    (pl0 * rk + (lo - b0)) * W1P,
                                    [[rk * W1P, np_], [W1P, hi - lo],
                                     [1, W1P]]))

                ra = r2pool.tile([128, 2 * RB2 + 3, W1P], F16, tag="ra")
                x2_read(ra, 0, 128, 0, 2 * j0, nin)
                rb = r2pool.tile([65, 2 * RB2 + 3, W1P], F16, tag="rb")
                x2_read(rb, 0, 65, 128, 2 * j0, nin)
                # T: rows shifted pair tile (lower = tail row r, upper = r+1)
                tt = r2pool.tile([128, 2 * RB2 + 3, W1P], F16, tag="tt")
                x2_read(tt, 0, 64, 128, 2 * j0, nin)
                nup = min(nin, cfg.r1 - (2 * j0 + 1))
                x2_read(tt, 64, 64, 128, 2 * j0 + 1, nup)

                def emit2(j, ps, _j0=j0):
                    qf = requant_full(ps, 128, 192, m2bc, scbc[:, 1:2],
                                      scbc[:, 4:5], scbc[:, 7:8],
                                      t2pool, "l2")
                    trp = tr2.tile([128, 2, 128], F16, tag="trp")
                    nc.tensor.transpose(trp[:, 0, :], qf[:, 0:128], ident)
                    nc.tensor.transpose(trp[0:64, 1, :], qf[:, 128:192], ident)
                    jj = _j0 + j
                    nc.scalar.copy(x3a[:, jj, 2:2 + cfg.w2o], trp[:, 0, :])
                    nc.scalar.copy(x3b[0:64, jj, 2:2 + cfg.w2o],
                                   trp[0:64, 1, :])

                ce = 2 * cfg.w2o - 1
                for j in range(nj):
                    ps = ps2.tile([128, 192], F32, tag="cps")
                    first = True
                    for ky in range(5):
                        for kx in range(5):
                            nc.tensor.matmul(
                                ps[:], ra[0:128, 2 * j + ky, kx:kx + ce:2],
                                w2a[:, ky * 5 + kx, :],
                                start=first, stop=False)
                            first = False
                    for kyp in range(2):
                        for kx in range(5):
                            nc.tensor.matmul(
                                ps[:],
                                tt[0:128, 2 * j + 2 * kyp, kx:kx + ce:2],
                                w2p[:, kyp * 5 + kx, :],
                                start=False, stop=False)
                    for kx in range(5):
                        nc.tensor.matmul(
                            ps[:], rb[0:65, 2 * j + 4, kx:kx + ce:2],
                            w2l[:, kx, :], start=False, stop=(kx == 4))
                    emit2(j, ps)

        # ===== L3/L4: weights-stationary (stationary = [K, cout] 1 free dim),
        # moving = activations with 2D pixel APs; output lands channel-major.
        def requant_cm(q_ap, cw, mcol, c5s_col, sclB_col, clp_col,
                       pool, tag, dims, out_writer):
            """Channel-major requant: q [cw, *dims] psum -> fp16 via writer."""
            s = pool.tile([128] + dims, F32, tag=tag + "s")
            sl = (slice(0, cw),) + tuple(slice(0, d) for d in dims)
            nc.scalar.activation(s[sl], q_ap,
                                 mybir.ActivationFunctionType.Identity,
                                 bias=half_col[0:cw, :], scale=mcol)
            if fast:
                v = pool.tile([128] + dims, F32, tag=tag + "v")
                nc.vector.tensor_scalar(v[sl], s[sl], clp_col, sclB_col,
                                        AOP.min, AOP.mult)
                ti = pool.tile([128] + dims, I32, tag=tag + "ti")
                nc.gpsimd.tensor_copy(ti[sl], v[sl])
                out_writer(ti[sl])
                return
            s2 = pool.tile([128] + dims, F32, tag=tag + "s2")
            nc.vector.tensor_scalar(s2[sl], s[sl], 0.0, clp_col,
                                    AOP.max, AOP.min)
            ti1 = pool.tile([128] + dims, I32, tag=tag + "ti1")
            nc.gpsimd.tensor_copy(ti1[sl], s2[sl])
            g1 = pool.tile([128] + dims, F32, tag=tag + "g1")
            nc.gpsimd.tensor_tensor(g1[sl], ti1[sl], s2[sl], AOP.is_gt)
            c1 = pool.tile([128] + dims, F32, tag=tag + "c1")
            nc.vector.tensor_tensor(c1[sl], ti1[sl], g1[sl], AOP.subtract)
            v = pool.tile([128] + dims, F32, tag=tag + "v")
            nc.vector.tensor_scalar(v[sl], c1[sl], sclB_col, 0.5,
                                    AOP.mult, AOP.add)
            ti2 = pool.tile([128] + dims, I32, tag=tag + "ti2")
            nc.gpsimd.tensor_copy(ti2[sl], v[sl])
            g2 = pool.tile([128] + dims, F32, tag=tag + "g2")
            nc.gpsimd.tensor_tensor(g2[sl], ti2[sl], v[sl], AOP.is_gt)
            out_writer((ti2[sl], g2[sl]))

        def cm_write(dst_ap, res):
            if fast:
                nc.vector.tensor_copy(dst_ap, res)
            else:
                ti2, g2 = res
                nc.vector.tensor_tensor(dst_ap, ti2, g2, AOP.subtract)

        def conv_ws(wa, wp, wl, src_a, src_t, src_l, chunks, n_out_rows,
                    out_w, rpt, pspool, emit):
            # 25 full + 10 paired-tail + 5 leftover matmuls per psum
            j = 0
            while j < n_out_rows:
                jw = min(rpt, n_out_rows - j)
                for ci, (ca, cb) in enumerate(chunks):
                    cw = cb - ca
                    ps = pspool.tile([128, rpt, out_w], F32, tag="wps")
                    first = True
                    for ky in range(5):
                        for kx in range(5):
                            nc.tensor.matmul(
                                ps[:cw, :jw, :], wa[:, ky * 5 + kx, ca:cb],
                                src_a(j, jw, ky, kx), start=first, stop=False)
                            first = False
                    for kyp in range(2):
                        for kx in range(5):
                            nc.tensor.matmul(
                                ps[:cw, :jw, :], wp[:, kyp * 5 + kx, ca:cb],
                                src_t(j, jw, 2 * kyp, kx),
                                start=False, stop=False)
                    for kx in range(5):
                        nc.tensor.matmul(
                            ps[:cw, :jw, :], wl[:, kx, ca:cb],
                            src_l(j, jw, 4, kx), start=False, stop=(kx == 4))
                    emit(j, jw, ci, ca, cb, ps)
                j += jw

        # ============================ Layer 3 ============================
        RB3 = 8
        with tc.tile_pool(name="l3ps", bufs=4, space="PSUM") as ps3, \
             tc.tile_pool(name="l3t", bufs=2) as t3pool:

            # row-shifted tail pair tile for L3 (built after L2 completes)
            nc.sync.dma_start(out=t3t[0:64, :, :], in_=x3b[0:64, :, :])
            nc.sync.dma_start(out=t3t[64:128, 0:cfg.r2 - 1, :],
                              in_=x3b[0:64, 1:cfg.r2, :])

            def src_a3(j, jw, ky, kx):
                return x3a[0:128, 2 * j + ky:2 * j + ky + 2 * jw - 1:2,
                           kx:kx + 2 * cfg.w3o - 1:2]

            def src_t3(j, jw, ky, kx):
                return t3t[0:128, 2 * j + ky:2 * j + ky + 2 * jw - 1:2,
                           kx:kx + 2 * cfg.w3o - 1:2]

            def src_b3(j, jw, ky, kx):
                return x3b[0:65, 2 * j + ky:2 * j + ky + 2 * jw - 1:2,
                           kx:kx + 2 * cfg.w3o - 1:2]

            def emit3(j, jw, ci, ca, cb, ps):
                cw = cb - ca
                if ci == 0:
                    def wr(res, _j=j, _jw=jw):
                        cm_write(x4a[:, _j:_j + _jw, 2:2 + cfg.w3o], res)
                else:
                    def wr(res, _j=j, _jw=jw):
                        cm_write(x4b[0:64, _j:_j + _jw, 2:2 + cfg.w3o], res)
                requant_cm(ps[:cw, :jw, :], cw, m3sb[0:cw, ci:ci + 1],
                           scbc[0:cw, 8:9], scbc[0:cw, 5:6],
                           scbc[0:cw, 2:3], t3pool, "l3", [jw, cfg.w3o], wr)

            conv_ws(w3a, w3p, w3l, src_a3, src_t3, src_b3,
                    ((0, 128), (128, 192)), cfg.r3, cfg.w3o, RB3, ps3, emit3)

        # ============================ Layer 4 ============================
        with tc.tile_pool(name="l4ps", bufs=4, space="PSUM") as ps4, \
             tc.tile_pool(name="l4t", bufs=3) as t4pool:

            nc.sync.dma_start(out=t4t[0:64, :, :], in_=x4b[0:64, :, :])
            nc.sync.dma_start(out=t4t[64:128, 0:cfg.r3 - 1, :],
                              in_=x4b[0:64, 1:cfg.r3, :])

            def src_a4(j, jw, ky, kx):
                return x4a[0:128, 2 * j + ky:2 * j + ky + 2 * jw - 1:2,
                           kx:kx + 2 * cfg.w4o - 1:2]

            def src_t4(j, jw, ky, kx):
                return t4t[0:128, 2 * j + ky:2 * j + ky + 2 * jw - 1:2,
                           kx:kx + 2 * cfg.w4o - 1:2]

            def src_b4(j, jw, ky, kx):
                return x4b[0:65, 2 * j + ky:2 * j + ky + 2 * jw - 1:2,
                           kx:kx + 2 * cfg.w4o - 1:2]

            def emit4(j, jw, ci, ca, cb, ps):
                cw = cb - ca
                npix = jw * cfg.w4o
                s = t4pool.tile([128, cfg.r4, cfg.w4o], F32, tag="l4s")
                nc.scalar.activation(s[:cw, :jw, :], ps[:cw, :jw, :],
                                     mybir.ActivationFunctionType.Identity,
                                     bias=0.0, scale=m4sb[0:cw, ci:ci + 1])
                of = t4pool.tile([128, cfg.r4, cfg.w4o], F32, tag="l4o")
                if fast:
                    ti1 = t4pool.tile([128, cfg.r4, cfg.w4o], I32, tag="l4ti")
                    nc.gpsimd.tensor_copy(ti1[:cw, :jw, :], s[:cw, :jw, :])
                    nc.vector.tensor_copy(of[:cw, :jw, :], ti1[:cw, :jw, :])
                else:
                    u = t4pool.tile([128, cfg.r4, cfg.w4o], F32, tag="l4u")
                    nc.vector.tensor_scalar(u[:cw, :jw, :], s[:cw, :jw, :],
                                            0.5, None, AOP.add)
                    ti1 = t4pool.tile([128, cfg.r4, cfg.w4o], I32, tag="l4ti")
                    nc.gpsimd.tensor_copy(ti1[:cw, :jw, :], u[:cw, :jw, :])
                    g1 = t4pool.tile([128, cfg.r4, cfg.w4o], F32, tag="l4g")
                    nc.gpsimd.tensor_tensor(g1[:cw, :jw, :], ti1[:cw, :jw, :],
                                            u[:cw, :jw, :], AOP.is_gt)
                    nc.vector.tensor_tensor(of[:cw, :jw, :], ti1[:cw, :jw, :],
                                            g1[:cw, :jw, :], AOP.subtract)
                nc.scalar.dma_start(
                    out=bass.AP(out_h,
                                ca * (cfg.r4 * cfg.w4o) + j * cfg.w4o,
                                [[cfg.r4 * cfg.w4o, cw], [1, npix]]),
                    in_=of[:cw, :jw, :])

            conv_ws(w4a, w4p, w4l, src_a4, src_t4, src_b4,
                    ((0, 128), (128, 256), (256, 320)),
                    cfg.r4, cfg.w4o, cfg.r4, ps4, emit4)

        consts_cm.__exit__(None, None, None)

    nc.finalize()
    return nc


# ======================= host-side preparation =======================

def host_prep(inputs, cfg: Cfg):
    x = np.asarray(inputs["x"], np.float32)
    relus = np.asarray(inputs["relus"], np.float32)
    Bits = int(np.asarray(inputs["Bits"]))

    WX = cfg.W + 4
    W1P = cfg.w1o + 4
    W2P = cfg.w2o + 4
    W3P = cfg.w3o + 4

    def wprep(w, b, cout):
        wq = np.round(np.asarray(w, np.float32)).astype(np.float16)
        bq = np.round(np.asarray(b, np.float32)).astype(np.float16)
        wt = np.transpose(wq, (1, 2, 3, 0))  # [cin, ky, kx, cout]
        a = np.ascontiguousarray(wt[0:128].reshape(128, 25, cout))
        bm = np.zeros((65, 25, cout), np.float16)
        bm[0:64] = wt[128:192].reshape(64, 25, cout)
        bm[64, 24, :] = bq
        return a, bm

    w2a, w2b = wprep(inputs["w2"], inputs["b2"], 192)
    # pair weights: [tail of tap (2k,kx) ; tail of tap (2k+1,kx)]
    w2p = np.zeros((128, 10, 192), np.float16)
    for kyp in range(2):
        for kx in range(5):
            w2p[0:64, kyp * 5 + kx, :] = w2b[0:64, (2 * kyp) * 5 + kx, :]
            w2p[64:128, kyp * 5 + kx, :] = w2b[0:64, (2 * kyp + 1) * 5 + kx, :]
    # leftover ky=4 taps (bias/mask row lives on tap (4,4))
    w2l = np.zeros((65, 5, 192), np.float16)
    for kx in range(5):
        w2l[:, kx, :] = w2b[:, 20 + kx, :]
    w3a, w3b = wprep(inputs["w3"], inputs["b3"], 192)
    w4a, w4b = wprep(inputs["w4"], inputs["b4"], 320)

    def pairs_of(wb, cout):
        wp = np.zeros((128, 10, cout), np.float16)
        for kyp in range(2):
            for kx in range(5):
                wp[0:64, kyp * 5 + kx] = wb[0:64, (2 * kyp) * 5 + kx]
                wp[64:128, kyp * 5 + kx] = wb[0:64, (2 * kyp + 1) * 5 + kx]
        wl = np.zeros((65, 5, cout), np.float16)
        for kx in range(5):
            wl[:, kx] = wb[:, 20 + kx]
        return wp, wl

    w3p, w3l = pairs_of(w3b, 192)
    w4p, w4l = pairs_of(w4b, 320)

    wq1 = np.round(np.asarray(inputs["w1"], np.float32))
    bq1 = np.round(np.asarray(inputs["b1"], np.float32))
    w1m = np.zeros((76, 192), np.float16)
    kx_perm = [0, 2, 4, 1, 3]
    for ky in range(5):
        for c in range(3):
            for slot, kx in enumerate(kx_perm):
                w1m[ky * 15 + c * 5 + slot, :] = wq1[:, c, ky, kx]
    w1m[75, :] = bq1

    m1B = (np.asarray(inputs["muls0"], np.float32) *
           np.float32(2.0 ** (-(19 + IN_SCALE - CLP_K))))
    m2B = np.asarray(inputs["muls1"], np.float32) * np.float32(2.0 ** -16)
    m3B = np.asarray(inputs["muls2"], np.float32) * np.float32(2.0 ** -16)
    m4B = np.asarray(inputs["muls3"], np.float32) * np.float32(2.0 ** -15)
    m1 = np.zeros((128, 2), np.float32)
    m1[:, 0] = m1B[0:128]
    m1[0:64, 1] = m1B[128:192]
    m3c = np.zeros((128, 2), np.float32)
    m3c[:, 0] = m3B[0:128]
    m3c[0:64, 1] = m3B[128:192]
    m4c = np.zeros((128, 3), np.float32)
    m4c[:, 0] = m4B[0:128]
    m4c[:, 1] = m4B[128:256]
    m4c[0:64, 2] = m4B[256:320]

    clp = np.round((np.float32(2.0 ** Bits - 1) / relus) *
                   np.float32(2.0 ** (16 + CLP_K))).astype(np.float32)
    scl = np.floor((relus + np.float32(2.0 ** 3)) /
                   np.float32(2.0 ** 4)).astype(np.float32)
    Bl = np.array([2.0 ** -20, 2.0 ** -16, 2.0 ** -16], np.float32)
    sc = np.zeros(12, np.float32)
    sc[0:3] = clp[0:3] + np.float32(0.5)
    sc[3:6] = scl[0:3] * np.float32(2.0 ** -19)
    sc[6:9] = (clp[0:3] + np.float32(0.5)) * Bl

    in_maps = []
    for core in range(N_CORES):
        n, h = core // 2, core % 2
        a4 = cfg.r4 * h
        def maskvec(nslots, base, full):
            j = np.arange(nslots)
            return (((base + j) >= 0) & ((base + j) < full)).astype(np.float16)

        mk1 = maskvec(cfg.r1, 8 * a4 - 14, cfg.fr1)
        mk2 = maskvec(cfg.r2, 4 * a4 - 6, cfg.fr2)
        mk3 = maskvec(cfg.r3, 2 * a4 - 2, cfg.fr3)

        xpad = np.zeros((3, 2 * cfg.rx_half + 4, WX + 4), np.float32)
        t0 = 16 * a4 - 30
        lo = max(0, -t0)
        hi = min(cfg.rx, cfg.H - t0)
        if hi > lo:
            xpad[:, lo:hi, 2:2 + cfg.W] = x[n, :, t0 + lo:t0 + hi, :]
        # 76-plane im2col in R partition order: plane q=(ky,c,s):
        # xR[q][i,u] = xpad[c, 2i+ky, 2u+kx(s)]; plane 75 = rowmask/256
        WH = WX // 2
        nflat = 76 * cfg.rx_half
        npad = ((nflat + 127) // 128) * 128
        xs = np.zeros((npad, WH), np.float32)
        kx_of = [0, 2, 4, 1, 3]
        for ky in range(5):
            for cc in range(3):
                for s in range(5):
                    kx = kx_of[s]
                    q = ky * 15 + cc * 5 + s
                    pl = xpad[cc, ky:ky + 2 * cfg.rx_half:2,
                              kx:kx + 2 * WH:2]
                    xs[q * cfg.rx_half:(q + 1) * cfg.rx_half] = pl
        xs[75 * cfg.rx_half:75 * cfg.rx_half + cfg.r1] = \
            (mk1[:, None].astype(np.float32) / 256.0)

        mk4 = maskvec(cfg.r4, a4, cfg.fr4)

        def plane(nin, wid, mo):
            p = np.zeros((nin, wid), np.float16)
            for i in range(nin):
                if (i - 4) % 2 == 0:
                    j = (i - 4) // 2
                    if 0 <= j < len(mo):
                        p[i, :] = mo[j]
            return p

        mp2 = plane(cfg.r1, W1P, mk2)
        mp3 = plane(cfg.r2, W2P, mk3)
        mp4 = plane(cfg.r3, W3P, mk4)

        in_maps.append({
            "x": xs, "w1m": w1m, "w2a": w2a, "w2p": w2p, "w2l": w2l,
            "w3a": w3a,
            "w3p": w3p, "w3l": w3l, "w4a": w4a, "w4p": w4p,
            "w4l": w4l, "m1": m1, "m2": m2B,
            "m3": m3c, "m4": m4c, "sc": sc, "mp2": mp2,
            "mp3": mp3, "mp4": mp4,
        })
    return in_maps


def assemble_output(results, cfg: Cfg):
    out = np.empty((4, 320, cfg.fr4, cfg.w4o), np.float32)
    for core in range(N_CORES):
        n, h = core // 2, core % 2
        r = np.asarray(results[core]["out"])
        r = r.reshape(320, cfg.r4, cfg.w4o)
        out[n, :, cfg.r4 * h:cfg.r4 * (h + 1), :] = r
    return out


def fast_safe(inputs):
    """Interval-arithmetic proof that the 'fast' rounding chain is exact
    for this data (all rounding inputs bounded away from tie cases)."""
    relus = np.asarray(inputs["relus"], np.float64)
    if not np.all(np.isfinite(relus)) or np.any(relus <= 0):
        return False
    scl = np.floor((relus + 8.0) / 16.0)
    if np.any(scl < 0) or np.any(scl > 1):
        return False
    A = 255.0
    for wk, bk, mk, B in (("w1", "b1", "muls0", 2.0 ** -20),
                          ("w2", "b2", "muls1", 2.0 ** -16),
                          ("w3", "b3", "muls2", 2.0 ** -16),
                          ("w4", "b4", "muls3", 2.0 ** -15)):
        wq = np.round(np.asarray(inputs[wk], np.float64))
        bq = np.round(np.asarray(inputs[bk], np.float64))
        Q = (float(np.abs(wq).reshape(wq.shape[0], -1).sum(1).max()) * A +
             float(np.abs(bq).max()))
        if Q >= 2 ** 23:
            return False
        t = Q * float(np.abs(np.asarray(inputs[mk], np.float64)).max()) * B
        if not t < 0.45:
            return False
        A = 0.0  # this layer's outputs are provably exactly 0
    return True


# ======================================================================
# fp8 fast path
#
# When a host-side interval proof (fp8_gate) certifies that
#   - every requant stage's pre-floor value is < 0.44 in magnitude (so all
#     activations after L1's requant are exactly 0, clips never bind, and
#     round-half-even == the reference floor chain), and
#   - round(w2..4) are small ints exactly representable in fp8e4,
# the network runs as:
#   - quant: round/clip via the +1.5*2^23 magic-add trick (2 ops, exact),
#     on the compact parity image (no im2col duplication on-device);
#     im2col expansion happens as cheap DRAM->DRAM shift DMAs.
#   - L1: f16 weights-stationary conv (96-cout chunks), requant = 2 ops
#     (x*M+magic; -magic) writing fp8 straight into the SBUF-resident
#     [96, 2, rows, cols] activation tile (no DRAM round trip).
#   - L2: fp8 DoubleRow act-stationary conv (2 cin-planes per matmul),
#     requant 2 ops, PE transpose to channel-major fp8.
#   - L3/L4: fp8 DoubleRow weights-stationary conv, channel-major
#     requant (2 ops) directly into the next layer's input tile.
# Bias never enters the matmuls: its requant contribution is provably
# absorbed (|b*M| < 0.44 -> rounds to 0), which the gate checks.
# ======================================================================

F8 = mybir.dt.float8e4
DRMODE = mybir.MatmulPerfMode.DoubleRow
MAGIC = np.float32(12582912.0)  # 1.5 * 2^23
ACT_IDENT = mybir.ActivationFunctionType.Identity


class FCfg:
    r1, r2, r3, r4 = 149, 73, 35, 16
    w1o, w2o, w3o, w4o = 256, 128, 64, 32
    X2R, X2W = 149, 272      # row-interleaved planes; Ko step 272 %16==0
    X3R, X3W = 73, 144       # Ko step 144 %16==0
    X4R, X4W = 35, 80        # Ko step 80 %16==0
    PR = 312                 # xc rows per (c,par) plane
    RB1 = 38                 # L1 block rows
    PL = 152 * 256           # xqr plane elems (256-wide: packed rows)


def fp8_gate(inputs):
    """Interval proof that the fp8/magic-rounding program is bit-exact
    for these inputs (all post-L1 activations are exactly 0)."""
    try:
        x = np.asarray(inputs["x"], np.float64)
        relus = np.asarray(inputs["relus"], np.float64)
        Bits = int(np.asarray(inputs["Bits"]))
        split = int(np.asarray(inputs["split"]))
    except Exception:
        return False
    if x.shape != (4, 3, 512, 512):
        return False
    if not np.isfinite(x).all() or x.min() < 0:
        return False
    if not np.isfinite(relus).all() or (relus <= 0).any():
        return False
    if Bits < 0 or split < 1:
        return False
    import ml_dtypes
    A = 256.0  # fp8(clip(.,255)) can round up to 256
    for wk, bk, mk, B, wcap in (("w1", "b1", "muls0", 2.0 ** -20, 440),
                                ("w2", "b2", "muls1", 2.0 ** -16, 16),
                                ("w3", "b3", "muls2", 2.0 ** -16, 16),
                                ("w4", "b4", "muls3", 2.0 ** -15, 16)):
        w = np.asarray(inputs[wk], np.float64)
        b = np.asarray(inputs[bk], np.float64)
        m = np.asarray(inputs[mk], np.float64)
        if not (np.isfinite(w).all() and np.isfinite(b).all()
                and np.isfinite(m).all()):
            return False
        wq = np.round(w)
        bq = np.round(b)
        if np.abs(wq).max() > wcap or np.abs(bq).max() > 2048:
            return False
        # the kernel convolves with fp8-rounded weights; bound with those.
        # L1 weights are pre-scaled by 2^-5 so each fp8 product stays in
        # e4m3 range (|w*x| <= 448); the 2^5 is folded back into M1.
        scale = 2.0 ** -5 if wk == "w1" else 1.0
        w8 = (wq * scale).astype(np.float32).astype(ml_dtypes.float8_e4m3)
        w8 = w8.astype(np.float64) / scale
        if not np.isfinite(w8).all():
            return False
        if np.abs(w8 * scale).max() > 240.0:  # fp8e4(IEEE) max finite
            return False
        Q = (float(np.abs(w8).reshape(w8.shape[0], -1).sum(1).max()) * A +
             float(np.abs(bq).max()))
        if Q >= 2 ** 23:
            return False
        if not (Q * float(np.abs(m).max()) * B < 0.44):
            return False
        A = 0.0
    return True


def build_fast():
    c = FCfg
    nc = bacc.Bacc("TRN2", target_bir_lowering=False, debug=False,
                   num_devices=N_CORES, detect_race_conditions=True)

    xc_h = nc.declare_dram_parameter("xc", [128, 15 * 258], F32,
                                     isOutput=False)
    w1_h = nc.declare_dram_parameter("w1m", [38, 2, 192], F8, isOutput=False)
    w2_h = nc.declare_dram_parameter("w2d", [96, 25, 2, 192], F8,
                                     isOutput=False)
    w3_h = nc.declare_dram_parameter("w3d", [96, 25, 2, 192], F8,
                                     isOutput=False)
    w4_h = nc.declare_dram_parameter("w4d", [96, 25, 2, 320], F8,
                                     isOutput=False)
    mc_h = nc.declare_dram_parameter("mc", [128, 10], F32, isOutput=False)
    m2_h = nc.declare_dram_parameter("m2", [192], F32, isOutput=False)
    out_h = nc.declare_dram_parameter("out", [320, c.r4 * c.w4o], F32,
                                      isOutput=True)
    xqc_h = nc.dram_tensor("xqc", [1920, 258], F8)
    xqr_h = nc.dram_tensor("xqr", [75, 152, 256], F8)

    with tile.TileContext(nc) as tc:
        consts_cm = tc.tile_pool(name="consts", bufs=1)
        consts = consts_cm.__enter__()

        ident = consts.tile([128, 128], F16)
        make_identity(nc, ident)
        mc = consts.tile([128, 10], F32, tag="mc")
        nc.scalar.dma_start(out=mc, in_=mc_h[:])
        m2bc = consts.tile([128, 2, 192], F32, tag="m2bc")
        nc.scalar.dma_start(out=m2bc,
                          in_=bass.AP(m2_h, 0, [[0, 128], [0, 2], [1, 192]]))
        w1sb = consts.tile([38, 2, 192], F8, tag="w1sb")
        nc.scalar.dma_start(out=w1sb, in_=w1_h[:])
        w2sb = consts.tile([96, 25, 2, 192], F8, tag="w2sb")
        w3sb = consts.tile([96, 25, 2, 192], F8, tag="w3sb")
        w4sb = consts.tile([96, 25, 2, 320], F8, tag="w4sb")

        x2sb = consts.tile([96, c.X2R, 2, c.X2W], F8, tag="x2sb")
        x3sb = consts.tile([96, c.X3R, 2, c.X3W], F8, tag="x3sb")
        x4sb = consts.tile([96, c.X4R, 2, c.X4W], F8, tag="x4sb")
        nc.vector.memset(x2sb[:, :, :, 0:2], 0.0)
        nc.vector.memset(x2sb[:, :, :, 258:260], 0.0)
        nc.gpsimd.memset(x3sb[:, :, :, 0:2], 0.0)
        nc.gpsimd.memset(x3sb[:, :, :, 130:132], 0.0)
        nc.vector.memset(x4sb[:, :, :, 0:2], 0.0)
        nc.vector.memset(x4sb[:, :, :, 66:68], 0.0)

        MGC = mc[:, 7:8]
        NMGC = mc[:, 8:9]

        # ---------------- quant + im2col expansion ----------------
        FW = 15 * 258
        HW2 = FW // 2
        with tc.tile_pool(name="quant", bufs=1) as qp:
            xcin = qp.tile([128, FW], F32, tag="xcin")
            tq = qp.tile([128, FW], F32, tag="tq")
            xq16 = qp.tile([128, FW], F8, tag="xq16")
            QC = FW // 4
            for hf, (c0, c1) in enumerate(
                    ((0, QC), (QC, 2 * QC), (2 * QC, 3 * QC), (3 * QC, FW))):
                dq = (nc.sync, nc.scalar, nc.gpsimd, nc.sync)[hf]
                dq2 = (nc.sync, nc.scalar, nc.gpsimd, nc.scalar)[hf]
                dq.dma_start(out=xcin[:, c0:c1],
                             in_=bass.AP(xc_h, c0, [[FW, 128], [1, c1 - c0]]))
                # half-scale quant: fp8e4 (IEEE) max finite is 240, so
                # store round(x*128) <= 128 and fold the 2x into M1
                nc.vector.tensor_scalar(tq[:, c0:c1], xcin[:, c0:c1],
                                        128.0, MGC, AOP.mult, AOP.add)
                nc.gpsimd.tensor_scalar(xq16[:, c0:c1], tq[:, c0:c1],
                                        MGC, 128.0, AOP.subtract, AOP.min)
                dq2.dma_start(
                    out=bass.AP(xqc_h, c0, [[FW, 128], [1, c1 - c0]]),
                    in_=xq16[:, c0:c1])
            # expansion: per (slot-shift s, channel cc) D2D DMA writing the
            # 5 ky planes. dim0 = row (152) keeps the modeled cost low.
            ei = 0
            for s in range(5):
                par, sp = (0, s) if s < 3 else (1, s - 3)
                for cc in range(3):
                    eng = (nc.sync, nc.scalar, nc.gpsimd)[ei % 3]
                    ei += 1
                    eng.dma_start(
                        out=bass.AP(xqr_h, (cc * 5 + s) * c.PL,
                                    [[256, 152], [15 * c.PL, 5], [1, 256]]),
                        in_=bass.AP(
                            xqc_h,
                            (cc * 2 + par) * c.PR * 258 + sp,
                            [[516, 152], [258, 5], [1, 256]]))

        # engine rotation helpers for the 2-op requant
        ENGS = (nc.vector, nc.gpsimd, nc.scalar)

        def requant2(k, ps_ap, t_tile, t_sl, out_ap, Mcol, pw):
            # pass1 reads PSUM: DVE/Act only. pass2 is SBUF->SBUF: Pool ok.
            ea = (nc.vector, nc.scalar)[k % 2]
            eb = (nc.gpsimd, nc.gpsimd, nc.gpsimd, nc.scalar)[k % 4]
            if ea is nc.scalar:
                nc.scalar.activation(t_tile[t_sl], ps_ap, ACT_IDENT,
                                     bias=MGC[0:pw, :], scale=Mcol)
            else:
                ea.tensor_scalar(t_tile[t_sl], ps_ap, Mcol, MGC[0:pw, :],
                                 AOP.mult, AOP.add)
            if eb is nc.scalar:
                nc.scalar.activation(out_ap, t_tile[t_sl], ACT_IDENT,
                                     bias=NMGC[0:pw, :], scale=1.0)
            else:
                eb.tensor_scalar(out_ap, t_tile[t_sl], MGC[0:pw, :], None,
                                 AOP.subtract)

        for tp in range(0, 25, 5):
            nc.scalar.dma_start(out=w2sb[:, tp:tp + 5], in_=w2_h[:, tp:tp + 5])

        # ----------- Layers 1-4: dataflow-interleaved emission -----------
        # Program order follows data dependencies (L1 block -> the L2 row
        # groups it unblocks -> the L3 tiles those unblock) so the tile
        # scheduler keeps the PE fed from the start.
        l2t_cm = tc.tile_pool(name="l2t", bufs=4)
        l2t = l2t_cm.__enter__()
        l2ps_cm = tc.tile_pool(name="l2ps", bufs=4, space="PSUM")
        l2ps = l2ps_cm.__enter__()
        rkc = [0]
        l3p = {}

        def emit_l1_block(rpool, l1ps, l1t, blk):
            j0 = blk * c.RB1
            nj = min(c.RB1, c.r1 - j0)
            R = rpool.tile([38, 2, c.RB1, 256], F8, tag="R")
            rq = nc.sync if blk % 2 == 0 else nc.scalar
            rq2 = nc.scalar if blk % 2 == 0 else nc.sync
            splits = (0, 8, 20, nj) if blk == 0 else (0, nj)
            for pl in range(2):
                for si in range(len(splits) - 1):
                    r0, r1 = splits[si], splits[si + 1]
                    (rq if pl == 0 else rq2).dma_start(
                        out=R[:, pl, r0:r1, :],
                        in_=bass.AP(xqr_h,
                                    pl * 37 * c.PL + (j0 + r0) * 256,
                                    [[c.PL, 38], [256, r1 - r0], [1, 256]]))
            for g in range(0, nj, 4):
                jw = min(4, nj - g)
                for ci in range(2):
                    Mcol = mc[0:96, ci:ci + 1]
                    ps = l1ps.tile([96, 4, 256], F32, tag="ps")
                    for hh in range(0, jw, 2):
                        hw_ = min(2, jw - hh)
                        nc.tensor.matmul(
                            ps[:, hh:hh + hw_, :],
                            w1sb[:, :, ci * 96:ci * 96 + 96],
                            R[:, 0:2, g + hh:g + hh + hw_, 0:256],
                            start=True, stop=True, perf_mode=DRMODE)
                    t = l1t.tile([96, 4, 256], F32, tag="t")
                    requant2(rkc[0], ps[:, :jw, :], t,
                             (slice(0, 96), slice(0, jw)),
                             x2sb[0:96, j0 + g:j0 + g + jw, ci, 2:258],
                             Mcol, 96)
                    rkc[0] += 1

        def emit_l2_group(jg):
            nr = min(2, c.r2 - jg)
            ps2 = l2ps.tile([128, 2, 192], F32, tag="ps2")
            for r in range(nr):
                j = jg + r
                for ky in range(5):
                    for kx in range(5):
                        tap = ky * 5 + kx
                        nc.tensor.matmul(
                            ps2[:, r, :],
                            x2sb[0:96, 2 * j + ky, 0:2, kx:kx + 255:2],
                            w2sb[0:96, tap, 0:2, :],
                            start=(tap == 0), stop=(tap == 24),
                            perf_mode=DRMODE)
            u2 = l2t.tile([128, 2, 192], F32, tag="u2")
            nc.vector.tensor_tensor(u2[:, :nr, :], ps2[:, :nr, :],
                                    m2bc[:, :nr, :], AOP.mult)
            # qh2 slots hold cout 96-chunks padded to 128 cols so the xbar
            # DMA transpose (in free %128) can do the channel-major turn;
            # pad cols transpose into partitions 96..127, never read.
            qh2 = l2t.tile([128, 2, 2, 128], F16, tag="qh2")
            for i in range(2):
                nc.gpsimd.tensor_scalar(qh2[:, :nr, i, 0:96],
                                        u2[:, :nr, i * 96:i * 96 + 96],
                                        MGC, MGC, AOP.add, AOP.subtract)
            xt = l2t.tile([128, 2, 2, 128], F16, tag="xt")
            for r in range(nr):
                for i in range(2):
                    eng = nc.scalar if (jg // 2 + r + i) % 2 == 0 else nc.sync
                    eng.dma_start(out=xt[:, r, i, :], in_=qh2[:, r, i, :],
                                  transpose=True)
            for i in range(2):
                nc.gpsimd.tensor_copy(x3sb[0:96, jg:jg + nr, i, 2:130],
                                      xt[0:96, 0:nr, i, 0:128])

        def emit_l3_tile(j0):
            l3ps, l3t = l3p["ps"], l3p["t"]
            jw = min(8, c.r3 - j0)
            for ci in range(2):
                ps3 = l3ps.tile([96, 8, 64], F32, tag="ps3")
                for ky in range(5):
                    for kx in range(5):
                        tap = ky * 5 + kx
                        nc.tensor.matmul(
                            ps3[:, :jw, :],
                            w3sb[0:96, tap, 0:2, ci * 96:ci * 96 + 96],
                            x3sb[0:96,
                                 2 * j0 + ky:2 * j0 + ky + 2 * jw - 1:2,
                                 0:2, kx:kx + 127:2].rearrange(
                                     "k r t c -> k t r c"),
                            start=(tap == 0), stop=(tap == 24),
                            perf_mode=DRMODE)
                t3 = l3t.tile([96, 8, 64], F32, tag="t3")
                requant2(rkc[0], ps3[:, :jw, :], t3,
                         (slice(0, 96), slice(0, jw)),
                         x4sb[0:96, j0:j0 + jw, ci, 2:66],
                         mc[0:96, 2 + ci:3 + ci], 96)
                rkc[0] += 1

        def emit_l4_half(l4ps, l4t, j0h, jh):
            for ci, (ca, cb) in enumerate(((0, 128), (128, 256), (256, 320))):
                cw = cb - ca
                ps4 = l4ps.tile([128, 8, 32], F32, tag="ps4")
                for ky in range(5):
                    for kx in range(5):
                        tap = ky * 5 + kx
                        nc.tensor.matmul(
                            ps4[:cw, :jh, :],
                            w4sb[0:96, tap, 0:2, ca:cb],
                            x4sb[0:96,
                                 2 * j0h + ky:2 * j0h + ky + 2 * jh - 1:2,
                                 0:2, kx:kx + 63:2].rearrange(
                                     "k r t c -> k t r c"),
                            start=(tap == 0), stop=(tap == 24),
                            perf_mode=DRMODE)
                t4 = l4t.tile([128, 8, 32], F32, tag="t4")
                ea = (nc.vector, nc.scalar, nc.vector)[ci]
                if ea is nc.scalar:
                    nc.scalar.activation(t4[:cw, :jh], ps4[:cw, :jh],
                                         ACT_IDENT, bias=MGC[0:cw, :],
                                         scale=mc[0:cw, 4 + ci:5 + ci])
                else:
                    ea.tensor_scalar(t4[:cw, :jh], ps4[:cw, :jh],
                                     mc[0:cw, 4 + ci:5 + ci], MGC[0:cw, :],
                                     AOP.mult, AOP.add)
                of = l4t.tile([128, 8, 32], F32, tag="of")
                nc.gpsimd.tensor_scalar(of[:cw, :jh], t4[:cw, :jh],
                                        MGC[0:cw, :], None, AOP.subtract)
                oq = (nc.sync, nc.scalar, nc.sync)[ci]
                oq.dma_start(
                    out=bass.AP(out_h, ca * 512 + j0h * 32,
                                [[512, cw], [1, jh * 32]]),
                    in_=of[:cw, :jh])

        with tc.tile_pool(name="l1r", bufs=2) as rpool, \
             tc.tile_pool(name="l1ps", bufs=2, space="PSUM") as l1ps, \
             tc.tile_pool(name="l1t", bufs=3) as l1t:
            for blk in range(4):
                if blk == 2:
                    # L3/L4 weights: small chunks sized to the idle slots
                    # before the expansion burst; w3 on Pool so any spill
                    # can't delay the R0 gather queues (sync/scalar)
                    for tp in range(0, 25, 3):
                        te = min(tp + 3, 25)
                        nc.sync.dma_start(out=w3sb[:, tp:te],
                                          in_=w3_h[:, tp:te])
                        nc.sync.dma_start(out=w4sb[:, tp:te],
                                          in_=w4_h[:, tp:te])
                emit_l1_block(rpool, l1ps, l1t, blk)

        for jg in range(0, 38, 2):
            emit_l2_group(jg)

        with tc.tile_pool(name="l3ps", bufs=2, space="PSUM") as l3ps, \
             tc.tile_pool(name="l3t", bufs=2) as l3t:
            l3p["ps"], l3p["t"] = l3ps, l3t
            emit_l3_tile(0)                     # needs x3sb rows <= 18
            for jg in range(38, 54, 2):
                emit_l2_group(jg)
            emit_l3_tile(8)                     # rows <= 34
            emit_l3_tile(16)                    # rows <= 50
            with tc.tile_pool(name="l4ps", bufs=2, space="PSUM") as l4ps, \
                 tc.tile_pool(name="l4t", bufs=3) as l4t:
                emit_l4_half(l4ps, l4t, 0, 8)   # x4sb rows <= 18
                for jg in range(54, c.r2, 2):
                    emit_l2_group(jg)
                emit_l3_tile(24)
                emit_l3_tile(32)
                emit_l4_half(l4ps, l4t, 8, 8)

        l2ps_cm.__exit__(None, None, None)
        l2t_cm.__exit__(None, None, None)

        consts_cm.__exit__(None, None, None)

    nc.finalize()
    return nc


def host_prep_fast(inputs):
    import ml_dtypes
    c = FCfg
    F8NP = ml_dtypes.float8_e4m3
    x = np.asarray(inputs["x"], np.float32)

    wq1 = np.round(np.asarray(inputs["w1"], np.float32))
    w1f = np.zeros((76, 192), np.float32)
    for ky in range(5):
        for cc in range(3):
            for s in range(5):
                kx = 2 * s if s < 3 else 2 * (s - 3) + 1
                w1f[ky * 15 + cc * 5 + s, :] = wq1[:, cc, ky, kx]
    w1p = np.zeros((38, 2, 192), np.float32)
    w1p[:, 0, :] = w1f[0:38]
    w1p[1:38, 1, :] = w1f[38:75]   # w1p[0,1] stays 0: plane 37 dup guard
    w1m = (w1p * np.float32(2.0 ** -5)).astype(F8NP)

    def wdr(wk, cout):
        wq = np.round(np.asarray(inputs[wk], np.float32))
        # [cout, 192, 5, 5] -> [96, 25, 2, cout]
        arr = np.transpose(wq.reshape(cout, 2, 96, 25), (2, 3, 1, 0))
        return np.ascontiguousarray(arr).astype(F8NP)

    w2d = wdr("w2", 192)
    w3d = wdr("w3", 192)
    w4d = wdr("w4", 320)

    m1B = np.asarray(inputs["muls0"], np.float32) * np.float32(2.0 ** -14)
    m2B = np.asarray(inputs["muls1"], np.float32) * np.float32(2.0 ** -16)
    m3B = np.asarray(inputs["muls2"], np.float32) * np.float32(2.0 ** -16)
    m4B = np.asarray(inputs["muls3"], np.float32) * np.float32(2.0 ** -15)
    mc = np.zeros((128, 10), np.float32)
    mc[0:96, 0] = m1B[0:96]
    mc[0:96, 1] = m1B[96:192]
    mc[0:96, 2] = m3B[0:96]
    mc[0:96, 3] = m3B[96:192]
    mc[:, 4] = m4B[0:128]
    mc[:, 5] = m4B[128:256]
    mc[0:64, 6] = m4B[256:320]
    mc[:, 7] = MAGIC
    mc[:, 8] = -MAGIC

    in_maps = []
    for core in range(N_CORES):
        n, h = core // 2, core % 2
        a4 = c.r4 * h
        xpad = np.zeros((3, 307, 516), np.float32)
        t0 = 16 * a4 - 30
        lo = max(0, -t0)
        hi = min(307, 512 - t0)
        if hi > lo:
            xpad[:, lo:hi, 2:514] = x[n, :, t0 + lo:t0 + hi, :]
        xp = np.zeros((3, 2, c.PR, 258), np.float32)
        xp[:, 0, 0:307, :] = xpad[:, :, 0::2]
        xp[:, 1, 0:307, :] = xpad[:, :, 1::2]
        flat = np.zeros((1920, 258), np.float32)
        flat[0:1872] = xp.reshape(1872, 258)
        in_maps.append({
            "xc": np.ascontiguousarray(flat.reshape(128, 15 * 258)),
            "w1m": w1m, "w2d": w2d, "w3d": w3d, "w4d": w4d,
            "mc": mc, "m2": m2B,
        })
    return in_maps


def assemble_fast(results):
    c = FCfg
    out = np.empty((4, 320, 32, 32), np.float32)
    for core in range(N_CORES):
        n, h = core // 2, core % 2
        r = np.asarray(results[core]["out"]).reshape(320, c.r4, c.w4o)
        out[n, :, c.r4 * h:c.r4 * (h + 1), :] = r
    return out


_cached = {}


def _get_nc(fast):
    key = "fast" if fast else "exact"
    if key not in _cached:
        _cached[key] = build_program(Cfg(H=512, W=512, rows4=16), fast=fast)
    return _cached[key]


def _get_nc_fp8():
    if "fp8" not in _cached:
        _cached["fp8"] = build_fast()
    return _cached["fp8"]


def kernel(**inputs) -> np.ndarray:
    if fp8_gate(inputs):
        nc = _get_nc_fp8()
        in_maps = host_prep_fast(inputs)
        res = run_bass_kernel_spmd(nc, in_maps, core_ids=list(range(N_CORES)))
        return assemble_fast(res.results)
    cfg = Cfg(H=512, W=512, rows4=16)
    nc = _get_nc(fast_safe(inputs))
    in_maps = host_prep(inputs, cfg)
    res = run_bass_kernel_spmd(nc, in_maps, core_ids=list(range(N_CORES)))
    return assemble_output(res.results, cfg)


def run_traced(**inputs):
    if fp8_gate(inputs):
        nc = build_fast()
        in_maps = host_prep_fast(inputs)
        res = run_bass_kernel_spmd(nc, in_maps,
                                   core_ids=list(range(N_CORES)), trace=True)
        return assemble_fast(res.results), res
    cfg = Cfg(H=512, W=512, rows4=16)
    nc = build_program(cfg, fast=fast_safe(inputs))
    in_maps = host_prep(inputs, cfg)
    res = run_bass_kernel_spmd(nc, in_maps, core_ids=list(range(N_CORES)),
                               trace=True)
    return assemble_output(res.results, cfg), res

